# revision 4
# baseline (speedup 1.0000x reference)
"""DeepChebNet (3-layer ChebConv K=3 + MLP head) on 8 Trainium2 NeuronCores.

Strategy (1D node partition per the sharding hint):
  - 50000 nodes padded to 51200, split into two 25600-row half-tables
    (int16 gather index limit). Each core owns 3200 nodes of each half
    (6400 total = 50 x 128-node blocks of dst).
  - Edges are assigned to the core/block owning their dst and grouped by
    (src half, dst block); each group is padded to t_half 128-edge tiles
    (t_half = global max so the SPMD program is uniform across cores).
  - Each propagate runs two passes (src half A then B). Gathers are
    SWDGE dma_gather calls of 8 tiles (1024 descs) rotating across all
    4 queues with a 4096-desc ring (dynamic_dma_scratch_size=65536):
    ~2.25 ns/desc aggregate vs ~7.9 ns/desc single-queue.
  - Selection matrices are built ON-CHIP per tile with one DVE
    tensor_scalar op: S[e, j] = (iota_j == dst_e) * norm_e. This
    removes the 29.5 MB/propagate smat DRAM stream entirely.
  - Pass A accumulates each block in PSUM and parks it in an f32 SBUF
    partial; pass B adds its PSUM, applies the Chebyshev recurrence,
    and (fused, per block) the ChebConv output matmuls + bias/ReLU,
    node-major table rebuild (PE transpose) and the MLP head, so the
    lo-half AllGather of the next table fires mid-propagate and
    overlaps the remaining hi-half work.
"""
import numpy as np

import concourse.bacc as bacc
import concourse.bass as bass
import concourse.mybir as mybir
import concourse.tile as tile
from concourse.bass_utils import run_bass_kernel_spmd
from concourse.masks import make_identity

# problem constants (hardcoded per harness contract)
N_NODES = 50000
N_EDGES = 800000
D = 128
K = 3
BN_EPS = 1e-5

N_CORES = 8
P = 128
N_PAD = 51200
HALF = 25600            # rows per half-table (< 32768: int16-indexable)
HSLAB = 3200            # per-core nodes per half
BLK_NODES = 6400        # per-core nodes
N_BLOCKS = 50           # per-core 128-node dst blocks
N_HB = 25               # lo-half dst blocks per core
TILES_PER_CALL = 8      # 1024 descs/call <= 4096-desc ring
N_QUEUES = 4
SCRATCH = 65536         # SWDGE ring: 4096 descs/queue

F16 = mybir.dt.float16
F32 = mybir.dt.float32
npf16 = np.float16


def _owner_block(n):
    """global node id -> (core, block 0..49) under the lo/hi layout."""
    lo = n < HALF
    core = np.where(lo, n // HSLAB, (n - HALF) // HSLAB)
    blk = np.where(lo, (n % HSLAB) // P, N_HB + ((n - HALF) % HSLAB) // P)
    return core, blk


def _preprocess(edge_index, edge_weight):
    """Graph partition -> per-core gather idx stream + per-tile dst/norm."""
    src = np.asarray(edge_index[0], dtype=np.int64)
    dst = np.asarray(edge_index[1], dtype=np.int64)
    w = np.asarray(edge_weight, dtype=np.float32)

    deg = np.bincount(src, weights=w.astype(np.float64), minlength=N_NODES)
    deg = deg.astype(np.float32)
    degs = np.sqrt(np.maximum(deg, 1e-38))
    dinv = np.where(deg > 0, 1.0 / degs, 0.0).astype(np.float32)
    norm = (-dinv[src] * w * dinv[dst]).astype(np.float32)

    core, blk = _owner_block(dst)
    half = (src >= HALF).astype(np.int64)
    # stream order per core: pass A (src half 0) blocks 0..49, then pass B
    key = (core * 2 + half) * N_BLOCKS + blk
    order = np.argsort(key, kind="stable")
    src_s, dst_s, norm_s, key_s = src[order], dst[order], norm[order], key[order]
    dstl_s = (dst_s % P).astype(np.float32)

    n_groups = N_CORES * 2 * N_BLOCKS    # 800 (core, half, block) groups
    bounds = np.searchsorted(key_s, np.arange(n_groups + 1))
    counts = bounds[1:] - bounds[:-1]
    # groups of pad dst nodes (>= N_NODES) are empty; their tiles are all-pad
    t_half = max(1, int(np.max((counts + P - 1) // P)))  # tiles per group

    T_h = N_BLOCKS * t_half        # tiles per half-stream
    T_tot = 2 * T_h
    slots_h = T_h * P

    idx_all, dstn_all = [], []
    for c in range(N_CORES):
        e_src = np.zeros(2 * slots_h, dtype=np.int16)
        e_dstl = np.zeros(2 * slots_h, dtype=np.float32)
        e_norm = np.zeros(2 * slots_h, dtype=np.float32)
        for h in range(2):
            for b in range(N_BLOCKS):
                g = (c * 2 + h) * N_BLOCKS + b
                lo, hi = bounds[g], bounds[g + 1]
                n = hi - lo
                base = (h * N_BLOCKS + b) * t_half * P
                e_src[base:base + n] = (src_s[lo:hi] - h * HALF).astype(np.int16)
                e_dstl[base:base + n] = dstl_s[lo:hi]
                e_norm[base:base + n] = norm_s[lo:hi]
        # idx stream: per gather call, flat slot i -> (row i%16, col i//16),
        # replicated across the 8 groups of 16 partitions.
        idx16 = np.zeros((P, 2 * slots_h // 16), dtype=np.int16)
        for h in range(2):
            t0 = 0
            while t0 < T_h:
                nt = min(TILES_PER_CALL, T_h - t0)
                s0 = h * slots_h + t0 * P
                s = e_src[s0:s0 + nt * P]
                arr = s.reshape(nt * P // 16, 16).T      # [16, ncols]
                for gs in range(8):
                    idx16[gs * 16:(gs + 1) * 16, s0 // 16:s0 // 16 + nt * P // 16] = arr
                t0 += nt
        idx_all.append(np.ascontiguousarray(idx16))
        dstn = np.zeros((P, 2 * T_tot), dtype=np.float32)
        dstn[:, :T_tot] = e_dstl.reshape(T_tot, P).T
        dstn[:, T_tot:] = e_norm.reshape(T_tot, P).T
        dstn_all.append(np.ascontiguousarray(dstn))
    return t_half, idx_all, dstn_all


def _build_program(t_half, b2_val):
    """Build the SPMD Bass program (identical across cores)."""
    nc = bacc.Bacc("TRN2", target_bir_lowering=False, debug=False,
                   num_devices=N_CORES, num_swdge_queues=N_QUEUES,
                   dynamic_dma_scratch_size=SCRATCH)

    T_h = N_BLOCKS * t_half
    T_tot = 2 * T_h
    slots_h = T_h * P
    calls = []                      # (h, t0, nt) in stream order
    for h in range(2):
        t0 = 0
        while t0 < T_h:
            nt = min(TILES_PER_CALL, T_h - t0)
            calls.append((h, t0, nt))
            t0 += nt

    # ---- I/O -----------------------------------------------------------
    xA = nc.dram_tensor("xA", [HALF, D], F16, kind="ExternalInput")
    xB = nc.dram_tensor("xB", [HALF, D], F16, kind="ExternalInput")
    x0fm = nc.dram_tensor("x0fm", [P, BLK_NODES], F16, kind="ExternalInput")
    idx_d = nc.dram_tensor("idx", [P, 2 * slots_h // 16], mybir.dt.int16,
                           kind="ExternalInput")
    dstn_d = nc.dram_tensor("dstn", [P, 2 * T_tot], F32, kind="ExternalInput")
    io_d = nc.dram_tensor("io", [P, P], F16, kind="ExternalInput")
    wts_d = nc.dram_tensor("wts", [P, 10 * D + 1], F16, kind="ExternalInput")
    bias_d = nc.dram_tensor("bias", [P, 4], F32, kind="ExternalInput")
    y_d = nc.dram_tensor("y", [1, BLK_NODES], F32, kind="ExternalOutput")

    tabsA = [nc.dram_tensor(f"tabA{i}", [HALF, D], F16, addr_space="Shared")
             for i in range(5)]
    tabsB = [nc.dram_tensor(f"tabB{i}", [HALF, D], F16, addr_space="Shared")
             for i in range(5)]
    rg = [list(range(N_CORES))]

    with tile.TileContext(nc) as tc:
        with (
            tc.tile_pool(name="const", bufs=1) as constp,
            tc.tile_pool(name="big", bufs=1) as bigp,
            tc.tile_pool(name="gat", bufs=12) as gatp,
            tc.tile_pool(name="sel", bufs=8) as selp,
            tc.tile_pool(name="nm", bufs=4) as nmp,
            tc.tile_pool(name="tmp", bufs=2) as tmpp,
            tc.tile_pool(name="ps", bufs=4, space="PSUM") as psp,
            tc.tile_pool(name="pst", bufs=2, space="PSUM") as pstp,
            tc.tile_pool(name="pso", bufs=2, space="PSUM") as psop,
            tc.tile_pool(name="dram", bufs=1, space="DRAM") as dramp,
        ):
            # ---- load constants -----------------------------------------
            idx_t = constp.tile([P, 2 * slots_h // 16], mybir.dt.int16)
            dstn_t = constp.tile([P, 2 * T_tot], F32)
            wts_t = constp.tile([P, 10 * D + 1], F16)
            bias_t = constp.tile([P, 4], F32)
            iota_t = constp.tile([P, P], F16)
            ident = constp.tile([P, P], F16)
            nc.sync.dma_start(idx_t[:], idx_d[:])
            nc.sync.dma_start(dstn_t[:], dstn_d[:])
            nc.sync.dma_start(wts_t[:], wts_d[:])
            nc.sync.dma_start(bias_t[:], bias_d[:])
            nc.sync.dma_start(iota_t[:], io_d[:])
            make_identity(nc, ident[:])

            def wslice(i):  # i-th [P, D] weight block (lhsT layout [fi, fo])
                return wts_t[:, i * D:(i + 1) * D]

            w2_ap = wts_t[:, 10 * D:10 * D + 1]

            # ---- feature-major activations [P, 6400] f16 + f32 partial --
            tA = bigp.tile([P, BLK_NODES], F16, tag="tA")
            tB = bigp.tile([P, BLK_NODES], F16, tag="tB")
            tC = bigp.tile([P, BLK_NODES], F16, tag="tC")
            tD = bigp.tile([P, BLK_NODES], F16, tag="tD")
            prt = bigp.tile([P, BLK_NODES], F32, tag="prt")
            nc.sync.dma_start(tA[:], x0fm[:])

            bncA = [dramp.tile([HSLAB, D], F16, tag=f"bncA{i}", name=f"bncA{i}")
                    for i in range(5)]
            bncB = [dramp.tile([HSLAB, D], F16, tag=f"bncB{i}", name=f"bncB{i}")
                    for i in range(5)]

            qctr = [0]

            def build_block_table(src_fm, blk, b, table):
                """PE-transpose one fm block into the node-major DRAM slab;
                fire the half AllGather when its last block lands."""
                blo, bhi, tabA_sh, tabB_sh = table
                pt = pstp.tile([P, P], F16, tag="pt", space="PSUM")
                nc.tensor.transpose(pt[:], src_fm[:, blk], ident[:])
                nm = nmp.tile([P, P], F16, tag="nm")
                nc.scalar.activation(nm[:], pt[:],
                                     mybir.ActivationFunctionType.Copy,
                                     scale=1.0)
                if b < N_HB:
                    nc.sync.dma_start(blo[b * P:(b + 1) * P, :], nm[:])
                else:
                    bb = b - N_HB
                    nc.sync.dma_start(bhi[bb * P:(bb + 1) * P, :], nm[:])
                if b == N_HB - 1:
                    nc.gpsimd.collective_compute(
                        "AllGather", mybir.AluOpType.bypass,
                        replica_groups=rg, ins=[blo[:]], outs=[tabA_sh[:]])
                elif b == N_BLOCKS - 1:
                    nc.gpsimd.collective_compute(
                        "AllGather", mybir.AluOpType.bypass,
                        replica_groups=rg, ins=[bhi[:]], outs=[tabB_sh[:]])

            def propagate(tabA_, tabB_, out_fm, tx0_fm=None, table=None,
                          cheb=None):
                """out_fm = A_hat @ table (feature-major, per dst block).
                If tx0_fm: out = 2*prop - tx0 (second Chebyshev step).
                If table=(blo,bhi,tabA,tabB): also emit the node-major table
                of out_fm, AllGathering each half as soon as it completes.
                cheb: dict(tx0, tx1, wbase, bias_col, relu, h_fm,
                table=None|(...), mlp=False) fused per-block in pass B."""
                gmap = {}
                state = {"next_call": 0, "covered": 0}

                def ensure(tg):
                    while tg >= state["covered"]:
                        h, t0, nt = calls[state["next_call"]]
                        g = gatp.tile([P, TILES_PER_CALL * P], F16, tag="g")
                        tab = tabA_ if h == 0 else tabB_
                        col0 = (h * slots_h + t0 * P) // 16
                        nc.gpsimd.dma_gather(
                            out_ap=g[:, :nt * P]
                                .rearrange("p (n d) -> p n d", d=D),
                            in_ap=tab[:],
                            idxs_ap=idx_t[:, col0:col0 + nt * P // 16],
                            num_idxs=nt * P,
                            num_idxs_reg=nt * P,
                            elem_size=D,
                            queue_num=qctr[0] % N_QUEUES,
                            single_packet=False,
                        )
                        qctr[0] += 1
                        for k in range(nt):
                            gmap[h * T_h + t0 + k] = (g, k)
                        state["next_call"] += 1
                        state["covered"] = h * T_h + t0 + nt

                for h in range(2):
                    for b in range(N_BLOCKS):
                        ps = psp.tile([P, P], F32, tag="ps", space="PSUM")
                        for k in range(t_half):
                            tg = h * T_h + b * t_half + k
                            ensure(tg)
                            g, off = gmap[tg]
                            s = selp.tile([P, P], F16, tag="s")
                            nc.vector.tensor_scalar(
                                out=s[:], in0=iota_t[:],
                                scalar1=dstn_t[:, tg:tg + 1],
                                scalar2=dstn_t[:, T_tot + tg:T_tot + tg + 1],
                                op0=mybir.AluOpType.is_equal,
                                op1=mybir.AluOpType.mult)
                            nc.tensor.matmul(
                                out=ps[:],
                                lhsT=g[:, off * P:(off + 1) * P],
                                rhs=s[:],
                                start=(k == 0), stop=(k == t_half - 1),
                            )
                        blk = slice(b * P, (b + 1) * P)
                        if h == 0:
                            nc.vector.tensor_copy(out=prt[:, blk], in_=ps[:])
                            continue
                        # ---- pass B: finalize block b -----------------
                        if tx0_fm is None:
                            nc.vector.tensor_tensor(
                                out=out_fm[:, blk], in0=ps[:],
                                in1=prt[:, blk], op=mybir.AluOpType.add)
                        else:
                            t1 = tmpp.tile([P, P], F32, tag="t1")
                            nc.vector.scalar_tensor_tensor(
                                out=t1[:], in0=prt[:, blk], scalar=2.0,
                                in1=tx0_fm[:, blk],
                                op0=mybir.AluOpType.mult,
                                op1=mybir.AluOpType.subtract)
                            nc.vector.scalar_tensor_tensor(
                                out=out_fm[:, blk], in0=ps[:], scalar=2.0,
                                in1=t1[:],
                                op0=mybir.AluOpType.mult,
                                op1=mybir.AluOpType.add)
                        if table is not None:
                            build_block_table(out_fm, blk, b, table)
                        if cheb is not None:
                            po = psop.tile([P, P], F32, tag="po",
                                           space="PSUM")
                            txs = (cheb["tx0"], cheb["tx1"], out_fm)
                            for k2, txk in enumerate(txs):
                                nc.tensor.matmul(
                                    out=po[:], lhsT=wslice(cheb["wbase"] + k2),
                                    rhs=txk[:, blk],
                                    start=(k2 == 0), stop=(k2 == 2))
                            h_fm = cheb["h_fm"]
                            bc = cheb["bias_col"]
                            if cheb["relu"]:
                                nc.scalar.activation(
                                    h_fm[:, blk], po[:],
                                    mybir.ActivationFunctionType.Relu,
                                    bias=bias_t[:, bc:bc + 1], scale=1.0)
                            else:
                                nc.vector.tensor_tensor(
                                    out=h_fm[:, blk], in0=po[:],
                                    in1=bias_t[:, bc:bc + 1]
                                        .broadcast_to([P, P]),
                                    op=mybir.AluOpType.add)
                            if cheb.get("table") is not None:
                                build_block_table(h_fm, blk, b, cheb["table"])
                            if cheb.get("mlp"):
                                pm = psop.tile([P, P], F32, tag="po",
                                               space="PSUM")
                                nc.tensor.matmul(out=pm[:], lhsT=wslice(9),
                                                 rhs=h_fm[:, blk],
                                                 start=True, stop=True)
                                h4 = nmp.tile([P, P], F16, tag="h4")
                                nc.scalar.activation(
                                    h4[:], pm[:],
                                    mybir.ActivationFunctionType.Relu,
                                    bias=bias_t[:, 3:4], scale=1.0)
                                p2 = psop.tile([1, P], F32, tag="po",
                                               space="PSUM")
                                nc.tensor.matmul(out=p2[:], lhsT=w2_ap,
                                                 rhs=h4[:],
                                                 start=True, stop=True)
                                yo = nmp.tile([1, P], F32, tag="yo")
                                nc.scalar.activation(
                                    yo[:], p2[:],
                                    mybir.ActivationFunctionType.Sigmoid,
                                    bias=b2_val, scale=1.0)
                                nc.sync.dma_start(y_d[:, blk], yo[:1, :])

            # ================= layer 1 =================
            propagate(xA, xB, tB,
                      table=(bncA[0], bncB[0], tabsA[0], tabsB[0]))  # Tx1
            propagate(tabsA[0], tabsB[0], tC, tx0_fm=tA,
                      cheb=dict(tx0=tA, tx1=tB, wbase=0, bias_col=0,
                                relu=True, h_fm=tD,
                                table=(bncA[1], bncB[1], tabsA[1], tabsB[1])))

            # ================= layer 2 =================
            propagate(tabsA[1], tabsB[1], tB,
                      table=(bncA[2], bncB[2], tabsA[2], tabsB[2]))
            propagate(tabsA[2], tabsB[2], tC, tx0_fm=tD,
                      cheb=dict(tx0=tD, tx1=tB, wbase=3, bias_col=1,
                                relu=True, h_fm=tA,
                                table=(bncA[3], bncB[3], tabsA[3], tabsB[3])))

            # ================= layer 3 + MLP head =================
            propagate(tabsA[3], tabsB[3], tB,
                      table=(bncA[4], bncB[4], tabsA[4], tabsB[4]))
            propagate(tabsA[4], tabsB[4], tC, tx0_fm=tA,
                      cheb=dict(tx0=tA, tx1=tB, wbase=6, bias_col=2,
                                relu=False, h_fm=tD, mlp=True))

    nc.finalize()
    return nc


_CACHE = {}


def kernel(x, edge_index, edge_weight, W_in, b_in, W_hid, b_hid, W_out, b_out,
           mlp_w1, mlp_b1, bn_gamma, bn_beta, bn_mean, bn_var, mlp_w2, mlp_b2,
           _trace=False):
    x = np.asarray(x, dtype=np.float32)
    t_half, idx_all, dstn_all = _preprocess(
        np.asarray(edge_index), np.asarray(edge_weight))

    b2_val = float(np.asarray(mlp_b2, np.float32).reshape(-1)[0])
    cache_key = (t_half, b2_val)
    if cache_key in _CACHE:
        nc = _CACHE[cache_key]
    else:
        nc = _build_program(t_half, b2_val)
        _CACHE[cache_key] = nc

    # ---- host-side tensor prep ----------------------------------------
    xpad = np.zeros((N_PAD, D), dtype=np.float32)
    xpad[:N_NODES] = x
    x16_np = xpad.astype(npf16)

    # BN folding: y = s*(h@W1 + b1) + t -> W1' = W1*s, b1' = b1*s + t
    s = (np.asarray(bn_gamma, np.float32)
         / np.sqrt(np.asarray(bn_var, np.float32) + BN_EPS))
    t_ = np.asarray(bn_beta, np.float32) - np.asarray(bn_mean, np.float32) * s
    w1p = np.asarray(mlp_w1, np.float32) * s[None, :]
    b1p = np.asarray(mlp_b1, np.float32) * s + t_

    wts = np.zeros((P, 10 * D + 1), dtype=npf16)
    for i, W in enumerate((W_in, W_hid, W_out)):
        W = np.asarray(W, np.float32)
        for k in range(K):
            wts[:, (i * K + k) * D:(i * K + k + 1) * D] = W[k].astype(npf16)
    wts[:, 9 * D:10 * D] = w1p.astype(npf16)
    wts[:, 10 * D:10 * D + 1] = np.asarray(mlp_w2, np.float32).astype(npf16)

    biases = np.zeros((P, 4), dtype=np.float32)
    biases[:, 0] = np.asarray(b_in, np.float32)
    biases[:, 1] = np.asarray(b_hid, np.float32)
    biases[:, 2] = np.asarray(b_out, np.float32)
    biases[:, 3] = b1p

    io = np.tile(np.arange(P, dtype=npf16), (P, 1))

    in_maps = []
    for c in range(N_CORES):
        own = np.concatenate([
            x16_np[c * HSLAB:(c + 1) * HSLAB],
            x16_np[HALF + c * HSLAB:HALF + (c + 1) * HSLAB],
        ], axis=0)  # [6400, 128]
        in_maps.append({
            "xA": x16_np[:HALF],
            "xB": x16_np[HALF:],
            "x0fm": np.ascontiguousarray(own.T),
            "idx": idx_all[c],
            "dstn": dstn_all[c],
            "io": io,
            "wts": wts,
            "bias": biases,
        })

    res = run_bass_kernel_spmd(nc, in_maps, list(range(N_CORES)), trace=_trace)
    y_full = np.zeros(N_PAD, dtype=np.float32)
    for c in range(N_CORES):
        yc = res.results[c]["y"][0]
        y_full[c * HSLAB:(c + 1) * HSLAB] = yc[:HSLAB]
        y_full[HALF + c * HSLAB:HALF + (c + 1) * HSLAB] = yc[HSLAB:]
    out = y_full[:N_NODES, None].astype(np.float32)
    if _trace:
        kernel._last_results = res
    return out


# revision 5
# speedup vs baseline: 1.0163x; 1.0163x over previous
"""DeepChebNet (3-layer ChebConv K=3 + MLP head) on 8 Trainium2 NeuronCores.

Strategy (1D node partition per the sharding hint):
  - 50000 nodes padded to 51200, split into two 25600-row half-tables
    (int16 gather index limit). Each core owns 3200 nodes of each half
    (6400 total = 50 x 128-node dst blocks). Tables are laid out
    chunk-major (5 chunks x 8 cores x 640 rows per half) so each
    half-table AllGather splits into 5 contiguous chunk-AllGathers that
    fire as soon as their 5 dst blocks finish — the halo exchange
    pipelines with compute instead of exposing a full-table tail.
  - Edges are assigned to the core/block owning their dst and grouped by
    (src half, dst block); each group is padded to t_half 128-edge tiles
    (t_half = global max so the SPMD program is uniform across cores).
  - Each propagate runs two passes (src half A then B), so pass A of the
    next propagate overlaps the hi-half AllGather of its input table.
    Gathers are SWDGE dma_gather calls of 8 tiles (1024 descs) rotating
    across all 4 queues with a 4096-desc ring
    (dynamic_dma_scratch_size=65536): ~2.3 ns/desc aggregate vs ~8
    ns/desc when a call overflows the default 1024-desc ring.
  - Per-block selection matrices (norm scattered by dst_local, fp16) are
    precomputed on host and streamed from DRAM; PE accumulates
    G^T @ S into PSUM (feature-major out).
  - Pass A parks each block's PSUM in an fp16 SBUF partial; pass B adds
    its PSUM, applies the Chebyshev recurrence, and (fused, per block)
    the ChebConv output matmuls + bias/ReLU, the PE-transpose table
    rebuild, and on the last layer the MLP head + sigmoid.
"""
import numpy as np

import concourse.bacc as bacc
import concourse.bass as bass
import concourse.mybir as mybir
import concourse.tile as tile
from concourse.bass_utils import run_bass_kernel_spmd
from concourse.masks import make_identity

# problem constants (hardcoded per harness contract)
N_NODES = 50000
N_EDGES = 800000
D = 128
K = 3
BN_EPS = 1e-5

N_CORES = 8
P = 128
N_PAD = 51200
HALF = 25600            # rows per half-table (< 32768: int16-indexable)
HSLAB = 3200            # per-core nodes per half
BLK_NODES = 6400        # per-core nodes
N_BLOCKS = 50           # per-core 128-node dst blocks
N_HB = 25               # lo-half dst blocks per core
N_CHUNKS = 5            # AllGather chunks per half-table
CBLK = N_HB // N_CHUNKS  # dst blocks per AG chunk (5)
CROWS = CBLK * P         # rows per core per chunk (640)
TILES_PER_CALL = 8      # 1024 descs/call <= 4096-desc ring
N_QUEUES = 4
SCRATCH = 65536         # SWDGE ring: 4096 descs/queue

F16 = mybir.dt.float16
F32 = mybir.dt.float32
npf16 = np.float16


def _owner_block(n):
    """global node id -> (core, block 0..49) under the lo/hi layout."""
    lo = n < HALF
    core = np.where(lo, n // HSLAB, (n - HALF) // HSLAB)
    blk = np.where(lo, (n % HSLAB) // P, N_HB + ((n - HALF) % HSLAB) // P)
    return core, blk


def _table_row(pos):
    """position within a half (core-major) -> chunk-major table row."""
    c, sp = pos // HSLAB, pos % HSLAB
    return (sp // CROWS) * (N_CORES * CROWS) + c * CROWS + sp % CROWS


def _chunk_perm_half(xh):
    """[HALF, D] core-major half -> chunk-major table layout."""
    return np.ascontiguousarray(
        xh.reshape(N_CORES, N_CHUNKS, CROWS, D)
          .transpose(1, 0, 2, 3).reshape(HALF, D))


def _preprocess(edge_index, edge_weight):
    """Graph partition -> per-core gather idx stream + selection matrices."""
    src = np.asarray(edge_index[0], dtype=np.int64)
    dst = np.asarray(edge_index[1], dtype=np.int64)
    w = np.asarray(edge_weight, dtype=np.float32)

    deg = np.bincount(src, weights=w.astype(np.float64), minlength=N_NODES)
    deg = deg.astype(np.float32)
    degs = np.sqrt(np.maximum(deg, 1e-38))
    dinv = np.where(deg > 0, 1.0 / degs, 0.0).astype(np.float32)
    norm = (-dinv[src] * w * dinv[dst]).astype(np.float32)

    core, blk = _owner_block(dst)
    half = (src >= HALF).astype(np.int64)
    # stream order per core: pass A (src half 0) blocks 0..49, then pass B
    key = (core * 2 + half) * N_BLOCKS + blk
    order = np.argsort(key, kind="stable")
    src_s, dst_s, norm_s, key_s = src[order], dst[order], norm[order], key[order]
    # chunk-major gather index within the half-table
    srcl_s = _table_row(src_s - half[order] * HALF).astype(np.int16)
    dstl_s = (dst_s % P).astype(np.int64)

    n_groups = N_CORES * 2 * N_BLOCKS    # 800 (core, half, block) groups
    bounds = np.searchsorted(key_s, np.arange(n_groups + 1))
    counts = bounds[1:] - bounds[:-1]
    # groups of pad dst nodes (>= N_NODES) are empty; their tiles are all-pad
    t_half = max(1, int(np.max((counts + P - 1) // P)))  # tiles per group

    T_h = N_BLOCKS * t_half        # tiles per half-stream
    T_tot = 2 * T_h
    slots_h = T_h * P

    idx_all, smat_all = [], []
    for c in range(N_CORES):
        e_src = np.zeros(2 * slots_h, dtype=np.int16)
        e_dstl = np.zeros(2 * slots_h, dtype=np.int64)
        e_norm = np.zeros(2 * slots_h, dtype=np.float32)
        e_live = np.zeros(2 * slots_h, dtype=bool)
        for h in range(2):
            for b in range(N_BLOCKS):
                g = (c * 2 + h) * N_BLOCKS + b
                lo, hi = bounds[g], bounds[g + 1]
                n = hi - lo
                base = (h * N_BLOCKS + b) * t_half * P
                e_src[base:base + n] = srcl_s[lo:hi]
                e_dstl[base:base + n] = dstl_s[lo:hi]
                e_norm[base:base + n] = norm_s[lo:hi]
                e_live[base:base + n] = True
        # idx stream: per gather call, flat slot i -> (row i%16, col i//16),
        # replicated across the 8 groups of 16 partitions.
        idx16 = np.zeros((P, 2 * slots_h // 16), dtype=np.int16)
        for h in range(2):
            t0 = 0
            while t0 < T_h:
                nt = min(TILES_PER_CALL, T_h - t0)
                s0 = h * slots_h + t0 * P
                s = e_src[s0:s0 + nt * P]
                arr = s.reshape(nt * P // 16, 16).T      # [16, ncols]
                for gs in range(8):
                    idx16[gs * 16:(gs + 1) * 16,
                          s0 // 16:s0 // 16 + nt * P // 16] = arr
                t0 += nt
        idx_all.append(np.ascontiguousarray(idx16))
        # selection matrices: smat[p, T*P + j] = norm_e if slot (tile T,
        # lane p) holds edge e with dst_local j
        slot = np.nonzero(e_live)[0]
        s_all = np.zeros(T_tot * P * P, dtype=npf16)
        s_all[slot * P + e_dstl[slot]] = e_norm[slot]
        smat = np.ascontiguousarray(
            s_all.reshape(T_tot, P, P).transpose(1, 0, 2).reshape(P, -1))
        smat_all.append(smat)
    return t_half, idx_all, smat_all


def _build_program(t_half, b2_val):
    """Build the SPMD Bass program (identical across cores)."""
    nc = bacc.Bacc("TRN2", target_bir_lowering=False, debug=False,
                   num_devices=N_CORES, num_swdge_queues=N_QUEUES,
                   dynamic_dma_scratch_size=SCRATCH)

    T_h = N_BLOCKS * t_half
    slots_h = T_h * P
    sw = t_half * P                 # smat cols per (half, block) unit
    calls = []                      # (h, t0, nt) in stream order
    for h in range(2):
        t0 = 0
        while t0 < T_h:
            nt = min(TILES_PER_CALL, T_h - t0)
            calls.append((h, t0, nt))
            t0 += nt

    # ---- I/O -----------------------------------------------------------
    xA = nc.dram_tensor("xA", [HALF, D], F16, kind="ExternalInput")
    xB = nc.dram_tensor("xB", [HALF, D], F16, kind="ExternalInput")
    x0fm = nc.dram_tensor("x0fm", [P, BLK_NODES], F16, kind="ExternalInput")
    idx_d = nc.dram_tensor("idx", [P, 2 * slots_h // 16], mybir.dt.int16,
                           kind="ExternalInput")
    smat_d = nc.dram_tensor("smat", [P, 2 * T_h * P], F16,
                            kind="ExternalInput")
    wts_d = nc.dram_tensor("wts", [P, 10 * D + 1], F16, kind="ExternalInput")
    bias_d = nc.dram_tensor("bias", [P, 4], F32, kind="ExternalInput")
    y_d = nc.dram_tensor("y", [1, BLK_NODES], F32, kind="ExternalOutput")

    tabsA = [nc.dram_tensor(f"tabA{i}", [HALF, D], F16, addr_space="Shared")
             for i in range(5)]
    tabsB = [nc.dram_tensor(f"tabB{i}", [HALF, D], F16, addr_space="Shared")
             for i in range(5)]
    rg = [list(range(N_CORES))]

    with tile.TileContext(nc) as tc:
        with (
            tc.tile_pool(name="const", bufs=1) as constp,
            tc.tile_pool(name="big", bufs=1) as bigp,
            tc.tile_pool(name="gat", bufs=12) as gatp,
            tc.tile_pool(name="sel", bufs=6) as selp,
            tc.tile_pool(name="nm", bufs=4) as nmp,
            tc.tile_pool(name="tmp", bufs=2) as tmpp,
            tc.tile_pool(name="ps", bufs=4, space="PSUM") as psp,
            tc.tile_pool(name="pst", bufs=2, space="PSUM") as pstp,
            tc.tile_pool(name="pso", bufs=2, space="PSUM") as psop,
            tc.tile_pool(name="dram", bufs=1, space="DRAM") as dramp,
        ):
            # ---- load constants -----------------------------------------
            idx_t = constp.tile([P, 2 * slots_h // 16], mybir.dt.int16)
            wts_t = constp.tile([P, 10 * D + 1], F16)
            bias_t = constp.tile([P, 4], F32)
            ident = constp.tile([P, P], F16)
            nc.sync.dma_start(idx_t[:], idx_d[:])
            nc.sync.dma_start(wts_t[:], wts_d[:])
            nc.sync.dma_start(bias_t[:], bias_d[:])
            make_identity(nc, ident[:])

            def wslice(i):  # i-th [P, D] weight block (lhsT layout [fi, fo])
                return wts_t[:, i * D:(i + 1) * D]

            w2_ap = wts_t[:, 10 * D:10 * D + 1]

            # ---- feature-major activations [P, 6400] f16 + f16 partial --
            tA = bigp.tile([P, BLK_NODES], F16, tag="tA")
            tB = bigp.tile([P, BLK_NODES], F16, tag="tB")
            tC = bigp.tile([P, BLK_NODES], F16, tag="tC")
            tD = bigp.tile([P, BLK_NODES], F16, tag="tD")
            prt = bigp.tile([P, BLK_NODES], F16, tag="prt")
            nc.sync.dma_start(tA[:], x0fm[:])

            bncA = [dramp.tile([HSLAB, D], F16, tag=f"bncA{i}", name=f"bncA{i}")
                    for i in range(5)]
            bncB = [dramp.tile([HSLAB, D], F16, tag=f"bncB{i}", name=f"bncB{i}")
                    for i in range(5)]

            qctr = [0]

            def build_block_table(src_fm, blk, b, table):
                """PE-transpose one fm block into the node-major DRAM slab;
                fire a chunk AllGather when its 5th block lands."""
                blo, bhi, tabA_sh, tabB_sh = table
                pt = pstp.tile([P, P], F16, tag="pt", space="PSUM")
                nc.tensor.transpose(pt[:], src_fm[:, blk], ident[:])
                nm = nmp.tile([P, P], F16, tag="nm")
                nc.scalar.activation(nm[:], pt[:],
                                     mybir.ActivationFunctionType.Copy,
                                     scale=1.0)
                bnc, bb, tab = (blo, b, tabA_sh) if b < N_HB else \
                               (bhi, b - N_HB, tabB_sh)
                nc.sync.dma_start(bnc[bb * P:(bb + 1) * P, :], nm[:])
                if bb % CBLK == CBLK - 1:
                    k = bb // CBLK
                    nc.gpsimd.collective_compute(
                        "AllGather", mybir.AluOpType.bypass,
                        replica_groups=rg,
                        ins=[bnc[k * CROWS:(k + 1) * CROWS, :]],
                        outs=[tab[k * N_CORES * CROWS:
                                  (k + 1) * N_CORES * CROWS, :]])

            def propagate(tabA_, tabB_, out_fm, tx0_fm=None, table=None,
                          cheb=None):
                """out_fm = A_hat @ table (feature-major, per dst block).
                If tx0_fm: out = 2*prop - tx0 (second Chebyshev step).
                table=(blo,bhi,tabA,tabB): emit node-major table of out_fm,
                chunk-AllGathering as blocks complete. cheb: dict(tx0, tx1,
                wbase, bias_col, relu, h_fm, table, mlp) fused in pass B."""
                gmap = {}
                state = {"next_call": 0, "covered": 0}

                def ensure(tg):
                    while tg >= state["covered"]:
                        h, t0, nt = calls[state["next_call"]]
                        g = gatp.tile([P, TILES_PER_CALL * P], F16, tag="g")
                        tab = tabA_ if h == 0 else tabB_
                        col0 = (h * slots_h + t0 * P) // 16
                        nc.gpsimd.dma_gather(
                            out_ap=g[:, :nt * P]
                                .rearrange("p (n d) -> p n d", d=D),
                            in_ap=tab[:],
                            idxs_ap=idx_t[:, col0:col0 + nt * P // 16],
                            num_idxs=nt * P,
                            num_idxs_reg=nt * P,
                            elem_size=D,
                            queue_num=qctr[0] % N_QUEUES,
                            single_packet=False,
                        )
                        qctr[0] += 1
                        for k in range(nt):
                            gmap[h * T_h + t0 + k] = (g, k)
                        state["next_call"] += 1
                        state["covered"] = h * T_h + t0 + nt

                for h in range(2):
                    for b in range(N_BLOCKS):
                        s = selp.tile([P, sw], F16, tag="s")
                        nc.scalar.dma_start(
                            s[:], smat_d[:, (h * N_BLOCKS + b) * sw:
                                         (h * N_BLOCKS + b + 1) * sw])
                        ps = psp.tile([P, P], F32, tag="ps", space="PSUM")
                        for k in range(t_half):
                            tg = h * T_h + b * t_half + k
                            ensure(tg)
                            g, off = gmap[tg]
                            nc.tensor.matmul(
                                out=ps[:],
                                lhsT=g[:, off * P:(off + 1) * P],
                                rhs=s[:, k * P:(k + 1) * P],
                                start=(k == 0), stop=(k == t_half - 1),
                            )
                        blk = slice(b * P, (b + 1) * P)
                        if h == 0:
                            nc.vector.tensor_copy(out=prt[:, blk], in_=ps[:])
                            continue
                        # ---- pass B: finalize block b -----------------
                        if tx0_fm is None:
                            nc.vector.tensor_tensor(
                                out=out_fm[:, blk], in0=ps[:],
                                in1=prt[:, blk], op=mybir.AluOpType.add)
                        else:
                            t1 = tmpp.tile([P, P], F32, tag="t1")
                            nc.vector.scalar_tensor_tensor(
                                out=t1[:], in0=prt[:, blk], scalar=2.0,
                                in1=tx0_fm[:, blk],
                                op0=mybir.AluOpType.mult,
                                op1=mybir.AluOpType.subtract)
                            nc.vector.scalar_tensor_tensor(
                                out=out_fm[:, blk], in0=ps[:], scalar=2.0,
                                in1=t1[:],
                                op0=mybir.AluOpType.mult,
                                op1=mybir.AluOpType.add)
                        if table is not None:
                            build_block_table(out_fm, blk, b, table)
                        if cheb is not None:
                            po = psop.tile([P, P], F32, tag="po",
                                           space="PSUM")
                            txs = (cheb["tx0"], cheb["tx1"], out_fm)
                            for k2, txk in enumerate(txs):
                                nc.tensor.matmul(
                                    out=po[:], lhsT=wslice(cheb["wbase"] + k2),
                                    rhs=txk[:, blk],
                                    start=(k2 == 0), stop=(k2 == 2))
                            h_fm = cheb["h_fm"]
                            bc = cheb["bias_col"]
                            if cheb["relu"]:
                                nc.scalar.activation(
                                    h_fm[:, blk], po[:],
                                    mybir.ActivationFunctionType.Relu,
                                    bias=bias_t[:, bc:bc + 1], scale=1.0)
                            else:
                                nc.vector.tensor_tensor(
                                    out=h_fm[:, blk], in0=po[:],
                                    in1=bias_t[:, bc:bc + 1]
                                        .broadcast_to([P, P]),
                                    op=mybir.AluOpType.add)
                            if cheb.get("table") is not None:
                                build_block_table(h_fm, blk, b, cheb["table"])
                            if cheb.get("mlp"):
                                pm = psop.tile([P, P], F32, tag="po",
                                               space="PSUM")
                                nc.tensor.matmul(out=pm[:], lhsT=wslice(9),
                                                 rhs=h_fm[:, blk],
                                                 start=True, stop=True)
                                h4 = nmp.tile([P, P], F16, tag="h4")
                                nc.scalar.activation(
                                    h4[:], pm[:],
                                    mybir.ActivationFunctionType.Relu,
                                    bias=bias_t[:, 3:4], scale=1.0)
                                p2 = psop.tile([1, P], F32, tag="po",
                                               space="PSUM")
                                nc.tensor.matmul(out=p2[:], lhsT=w2_ap,
                                                 rhs=h4[:],
                                                 start=True, stop=True)
                                yo = nmp.tile([1, P], F32, tag="yo")
                                nc.scalar.activation(
                                    yo[:], p2[:],
                                    mybir.ActivationFunctionType.Sigmoid,
                                    bias=b2_val, scale=1.0)
                                nc.sync.dma_start(y_d[:, blk], yo[:1, :])

            # ================= layer 1 =================
            propagate(xA, xB, tB,
                      table=(bncA[0], bncB[0], tabsA[0], tabsB[0]))  # Tx1
            propagate(tabsA[0], tabsB[0], tC, tx0_fm=tA,
                      cheb=dict(tx0=tA, tx1=tB, wbase=0, bias_col=0,
                                relu=True, h_fm=tD,
                                table=(bncA[1], bncB[1], tabsA[1], tabsB[1])))

            # ================= layer 2 =================
            propagate(tabsA[1], tabsB[1], tB,
                      table=(bncA[2], bncB[2], tabsA[2], tabsB[2]))
            propagate(tabsA[2], tabsB[2], tC, tx0_fm=tD,
                      cheb=dict(tx0=tD, tx1=tB, wbase=3, bias_col=1,
                                relu=True, h_fm=tA,
                                table=(bncA[3], bncB[3], tabsA[3], tabsB[3])))

            # ================= layer 3 + MLP head =================
            propagate(tabsA[3], tabsB[3], tB,
                      table=(bncA[4], bncB[4], tabsA[4], tabsB[4]))
            propagate(tabsA[4], tabsB[4], tC, tx0_fm=tA,
                      cheb=dict(tx0=tA, tx1=tB, wbase=6, bias_col=2,
                                relu=False, h_fm=tD, mlp=True))

    nc.finalize()
    return nc


_CACHE = {}


def kernel(x, edge_index, edge_weight, W_in, b_in, W_hid, b_hid, W_out, b_out,
           mlp_w1, mlp_b1, bn_gamma, bn_beta, bn_mean, bn_var, mlp_w2, mlp_b2,
           _trace=False):
    x = np.asarray(x, dtype=np.float32)
    t_half, idx_all, smat_all = _preprocess(
        np.asarray(edge_index), np.asarray(edge_weight))

    b2_val = float(np.asarray(mlp_b2, np.float32).reshape(-1)[0])
    cache_key = (t_half, b2_val)
    if cache_key in _CACHE:
        nc = _CACHE[cache_key]
    else:
        nc = _build_program(t_half, b2_val)
        _CACHE[cache_key] = nc

    # ---- host-side tensor prep ----------------------------------------
    xpad = np.zeros((N_PAD, D), dtype=np.float32)
    xpad[:N_NODES] = x
    x16_np = xpad.astype(npf16)
    xA_t = _chunk_perm_half(x16_np[:HALF])
    xB_t = _chunk_perm_half(x16_np[HALF:])

    # BN folding: y = s*(h@W1 + b1) + t -> W1' = W1*s, b1' = b1*s + t
    s = (np.asarray(bn_gamma, np.float32)
         / np.sqrt(np.asarray(bn_var, np.float32) + BN_EPS))
    t_ = np.asarray(bn_beta, np.float32) - np.asarray(bn_mean, np.float32) * s
    w1p = np.asarray(mlp_w1, np.float32) * s[None, :]
    b1p = np.asarray(mlp_b1, np.float32) * s + t_

    wts = np.zeros((P, 10 * D + 1), dtype=npf16)
    for i, W in enumerate((W_in, W_hid, W_out)):
        W = np.asarray(W, np.float32)
        for k in range(K):
            wts[:, (i * K + k) * D:(i * K + k + 1) * D] = W[k].astype(npf16)
    wts[:, 9 * D:10 * D] = w1p.astype(npf16)
    wts[:, 10 * D:10 * D + 1] = np.asarray(mlp_w2, np.float32).astype(npf16)

    biases = np.zeros((P, 4), dtype=np.float32)
    biases[:, 0] = np.asarray(b_in, np.float32)
    biases[:, 1] = np.asarray(b_hid, np.float32)
    biases[:, 2] = np.asarray(b_out, np.float32)
    biases[:, 3] = b1p

    in_maps = []
    for c in range(N_CORES):
        own = np.concatenate([
            x16_np[c * HSLAB:(c + 1) * HSLAB],
            x16_np[HALF + c * HSLAB:HALF + (c + 1) * HSLAB],
        ], axis=0)  # [6400, 128]
        in_maps.append({
            "xA": xA_t,
            "xB": xB_t,
            "x0fm": np.ascontiguousarray(own.T),
            "idx": idx_all[c],
            "smat": smat_all[c],
            "wts": wts,
            "bias": biases,
        })

    res = run_bass_kernel_spmd(nc, in_maps, list(range(N_CORES)), trace=_trace)
    y_full = np.zeros(N_PAD, dtype=np.float32)
    for c in range(N_CORES):
        yc = res.results[c]["y"][0]
        y_full[c * HSLAB:(c + 1) * HSLAB] = yc[:HSLAB]
        y_full[HALF + c * HSLAB:HALF + (c + 1) * HSLAB] = yc[HSLAB:]
    out = y_full[:N_NODES, None].astype(np.float32)
    if _trace:
        kernel._last_results = res
    return out


# revision 9
# speedup vs baseline: 1.0241x; 1.0077x over previous
"""DeepChebNet (3-layer ChebConv K=3 + MLP head) on 8 Trainium2 NeuronCores.

Strategy (1D node partition per the sharding hint):
  - 50000 nodes padded to 51200, split into two 25600-row half-tables
    (int16 gather index limit). Each core owns 3200 nodes of each half
    (6400 total = 50 x 128-node dst blocks). Tables are laid out
    chunk-major (5 chunks x 8 cores x 640 rows per half) so each
    half-table AllGather splits into 5 contiguous chunk-AllGathers.
  - Each propagate processes dst-half group G1 (blocks 0..24) fully,
    then G2 (blocks 25..49); within a group, pass A (src half lo) then
    pass B (src half hi). The lo-table chunk-AllGathers are emitted
    right after G1 (and hi after G2) so they trigger at ~55% / 100% of
    the propagate and their transfers pipeline behind compute; keeping
    them out of the gather stream means the Pool engine's SWDGE
    pipeline is never clamped to the compute frontier mid-propagate.
  - Gathers are SWDGE dma_gather calls of 16 tiles (2048 descs)
    rotating across all 4 queues with a 4096-desc ring
    (dynamic_dma_scratch_size=65536): ~2.3 ns/desc aggregate vs ~8
    ns/desc when a call overflows the default 1024-desc ring, with the
    1.5us/call fixed desc-gen overhead amortized over 2048 descs.
  - Per-(src half, block) selection matrices (norm scattered by
    dst_local, fp16) are precomputed on host and streamed from DRAM on
    both HWDGE queues (sync/scalar alternating); PE accumulates
    G^T @ S into PSUM (feature-major out).
  - Pass A parks each block's PSUM in an fp16 SBUF partial; pass B adds
    its PSUM, applies the Chebyshev recurrence, and (fused, per block)
    the ChebConv output matmuls + bias/ReLU, the PE-transpose table
    rebuild, and on the last layer the MLP head + sigmoid.
"""
import numpy as np

import concourse.bacc as bacc
import concourse.bass as bass
import concourse.mybir as mybir
import concourse.tile as tile
from concourse.bass_utils import run_bass_kernel_spmd
from concourse.masks import make_identity

# problem constants (hardcoded per harness contract)
N_NODES = 50000
N_EDGES = 800000
D = 128
K = 3
BN_EPS = 1e-5

N_CORES = 8
P = 128
N_PAD = 51200
HALF = 25600            # rows per half-table (< 32768: int16-indexable)
HSLAB = 3200            # per-core nodes per half
BLK_NODES = 6400        # per-core nodes
N_BLOCKS = 50           # per-core 128-node dst blocks
N_HB = 25               # dst blocks per half per core
N_CHUNKS = 5            # AllGather chunks per half-table
CBLK = N_HB // N_CHUNKS  # dst blocks per AG chunk (5)
CROWS = CBLK * P         # rows per core per chunk (640)
TILES_PER_CALL = 8      # 1024 descs/call (ucode row cap), 4 per 4096 ring
N_QUEUES = 4
SCRATCH = 65536         # SWDGE ring: 4096 descs/queue

F16 = mybir.dt.float16
F32 = mybir.dt.float32
npf16 = np.float16


def _owner_block(n):
    """global node id -> (core, block 0..49) under the lo/hi layout."""
    lo = n < HALF
    core = np.where(lo, n // HSLAB, (n - HALF) // HSLAB)
    blk = np.where(lo, (n % HSLAB) // P, N_HB + ((n - HALF) % HSLAB) // P)
    return core, blk


def _table_row(pos):
    """position within a half (core-major) -> chunk-major table row."""
    c, sp = pos // HSLAB, pos % HSLAB
    return (sp // CROWS) * (N_CORES * CROWS) + c * CROWS + sp % CROWS


def _chunk_perm_half(xh):
    """[HALF, D] core-major half -> chunk-major table layout."""
    return np.ascontiguousarray(
        xh.reshape(N_CORES, N_CHUNKS, CROWS, D)
          .transpose(1, 0, 2, 3).reshape(HALF, D))


def _preprocess(edge_index, edge_weight):
    """Graph partition -> per-core gather idx stream + selection matrices."""
    src = np.asarray(edge_index[0], dtype=np.int64)
    dst = np.asarray(edge_index[1], dtype=np.int64)
    w = np.asarray(edge_weight, dtype=np.float32)

    deg = np.bincount(src, weights=w.astype(np.float64), minlength=N_NODES)
    deg = deg.astype(np.float32)
    degs = np.sqrt(np.maximum(deg, 1e-38))
    dinv = np.where(deg > 0, 1.0 / degs, 0.0).astype(np.float32)
    norm = (-dinv[src] * w * dinv[dst]).astype(np.float32)

    core, blk = _owner_block(dst)
    half = (src >= HALF).astype(np.int64)
    dh = (blk >= N_HB).astype(np.int64)
    # stream order per core: dst-half group, then src half, then block
    key = (((core * 2 + dh) * 2 + half) * N_HB) + (blk - dh * N_HB)
    order = np.argsort(key, kind="stable")
    src_s, dst_s, norm_s, key_s = src[order], dst[order], norm[order], key[order]
    # chunk-major gather index within the src half-table
    srcl_s = _table_row(src_s - half[order] * HALF).astype(np.int16)
    dstl_s = (dst_s % P).astype(np.int64)

    n_groups = N_CORES * 4 * N_HB    # 800 (core, dh, half, block) groups
    bounds = np.searchsorted(key_s, np.arange(n_groups + 1))
    counts = bounds[1:] - bounds[:-1]
    # groups of pad dst nodes (>= N_NODES) are empty; their tiles are all-pad
    t_half = max(1, int(np.max((counts + P - 1) // P)))  # tiles per group

    T_seg = N_HB * t_half          # tiles per (dh, half) segment
    T_tot = 4 * T_seg
    sseg = T_seg * P

    idx_all, smat_all = [], []
    for c in range(N_CORES):
        e_src = np.zeros(4 * sseg, dtype=np.int16)
        e_dstl = np.zeros(4 * sseg, dtype=np.int64)
        e_norm = np.zeros(4 * sseg, dtype=np.float32)
        e_live = np.zeros(4 * sseg, dtype=bool)
        for seg in range(4):           # (dh, half)
            for b in range(N_HB):
                g = (c * 4 + seg) * N_HB + b
                lo, hi = bounds[g], bounds[g + 1]
                n = hi - lo
                base = (seg * N_HB + b) * t_half * P
                e_src[base:base + n] = srcl_s[lo:hi]
                e_dstl[base:base + n] = dstl_s[lo:hi]
                e_norm[base:base + n] = norm_s[lo:hi]
                e_live[base:base + n] = True
        # idx stream: per gather call, flat slot i -> (row i%16, col i//16),
        # replicated across the 8 groups of 16 partitions.
        idx16 = np.zeros((P, 4 * sseg // 16), dtype=np.int16)
        for seg in range(4):
            t0 = 0
            while t0 < T_seg:
                nt = min(TILES_PER_CALL, T_seg - t0)
                s0 = seg * sseg + t0 * P
                s = e_src[s0:s0 + nt * P]
                arr = s.reshape(nt * P // 16, 16).T      # [16, ncols]
                for gs in range(8):
                    idx16[gs * 16:(gs + 1) * 16,
                          s0 // 16:s0 // 16 + nt * P // 16] = arr
                t0 += nt
        idx_all.append(np.ascontiguousarray(idx16))
        # selection matrices: smat[p, T*P + j] = norm_e if slot (tile T,
        # lane p) holds edge e with dst_local j
        slot = np.nonzero(e_live)[0]
        s_all = np.zeros(4 * T_seg * P * P, dtype=npf16)
        s_all[slot * P + e_dstl[slot]] = e_norm[slot]
        smat = np.ascontiguousarray(
            s_all.reshape(4 * T_seg, P, P).transpose(1, 0, 2).reshape(P, -1))
        smat_all.append(smat)
    return t_half, idx_all, smat_all


def _build_program(t_half, b2_val):
    """Build the SPMD Bass program (identical across cores)."""
    nc = bacc.Bacc("TRN2", target_bir_lowering=False, debug=False,
                   num_devices=N_CORES, num_swdge_queues=N_QUEUES,
                   dynamic_dma_scratch_size=SCRATCH)

    T_seg = N_HB * t_half
    sseg = T_seg * P
    sw = t_half * P                 # smat cols per (seg, block) unit
    calls = []                      # (seg, t0, nt) in stream order
    for seg in range(4):
        t0 = 0
        while t0 < T_seg:
            nt = min(TILES_PER_CALL, T_seg - t0)
            calls.append((seg, t0, nt))
            t0 += nt

    # ---- I/O -----------------------------------------------------------
    xA = nc.dram_tensor("xA", [HALF, D], F16, kind="ExternalInput")
    xB = nc.dram_tensor("xB", [HALF, D], F16, kind="ExternalInput")
    x0fm = nc.dram_tensor("x0fm", [P, BLK_NODES], F16, kind="ExternalInput")
    idx_d = nc.dram_tensor("idx", [P, 4 * sseg // 16], mybir.dt.int16,
                           kind="ExternalInput")
    smat_d = nc.dram_tensor("smat", [P, 4 * T_seg * P], F16,
                            kind="ExternalInput")
    wts_d = nc.dram_tensor("wts", [P, 10 * D + 1], F16, kind="ExternalInput")
    bias_d = nc.dram_tensor("bias", [P, 4], F32, kind="ExternalInput")
    y_d = nc.dram_tensor("y", [1, BLK_NODES], F32, kind="ExternalOutput")

    tabsA = [nc.dram_tensor(f"tabA{i}", [HALF, D], F16, addr_space="Shared")
             for i in range(5)]
    tabsB = [nc.dram_tensor(f"tabB{i}", [HALF, D], F16, addr_space="Shared")
             for i in range(5)]
    rg = [list(range(N_CORES))]

    with tile.TileContext(nc) as tc:
        with (
            tc.tile_pool(name="const", bufs=1) as constp,
            tc.tile_pool(name="big", bufs=1) as bigp,
            tc.tile_pool(name="gat", bufs=8) as gatp,
            tc.tile_pool(name="sel", bufs=6) as selp,
            tc.tile_pool(name="nm", bufs=4) as nmp,
            tc.tile_pool(name="tmp", bufs=2) as tmpp,
            tc.tile_pool(name="ps", bufs=4, space="PSUM") as psp,
            tc.tile_pool(name="pst", bufs=2, space="PSUM") as pstp,
            tc.tile_pool(name="pso", bufs=2, space="PSUM") as psop,
            tc.tile_pool(name="dram", bufs=1, space="DRAM") as dramp,
        ):
            # ---- load constants -----------------------------------------
            idx_t = constp.tile([P, 4 * sseg // 16], mybir.dt.int16)
            wts_t = constp.tile([P, 10 * D + 1], F16)
            bias_t = constp.tile([P, 4], F32)
            ident = constp.tile([P, P], F16)
            nc.sync.dma_start(idx_t[:], idx_d[:])
            nc.sync.dma_start(wts_t[:], wts_d[:])
            nc.sync.dma_start(bias_t[:], bias_d[:])
            make_identity(nc, ident[:])

            def wslice(i):  # i-th [P, D] weight block (lhsT layout [fi, fo])
                return wts_t[:, i * D:(i + 1) * D]

            w2_ap = wts_t[:, 10 * D:10 * D + 1]

            # ---- feature-major activations [P, 6400] f16 + f16 partial --
            tA = bigp.tile([P, BLK_NODES], F16, tag="tA")
            tB = bigp.tile([P, BLK_NODES], F16, tag="tB")
            tC = bigp.tile([P, BLK_NODES], F16, tag="tC")
            tD = bigp.tile([P, BLK_NODES], F16, tag="tD")
            prt = bigp.tile([P, N_HB * P], F16, tag="prt")
            nc.sync.dma_start(tA[:], x0fm[:])

            bncA = [dramp.tile([HSLAB, D], F16, tag=f"bncA{i}", name=f"bncA{i}")
                    for i in range(5)]
            bncB = [dramp.tile([HSLAB, D], F16, tag=f"bncB{i}", name=f"bncB{i}")
                    for i in range(5)]

            qctr = [0]
            sel_q = [0]

            def build_block_table(src_fm, blk, b, table, pending):
                """PE-transpose one fm block into the node-major DRAM slab;
                queue a chunk AllGather when its 5th block lands."""
                blo, bhi, tabA_sh, tabB_sh = table
                pt = pstp.tile([P, P], F16, tag="pt", space="PSUM")
                nc.tensor.transpose(pt[:], src_fm[:, blk], ident[:])
                nm = nmp.tile([P, P], F16, tag="nm")
                nc.scalar.activation(nm[:], pt[:],
                                     mybir.ActivationFunctionType.Copy,
                                     scale=1.0)
                bnc, bb, tab = (blo, b, tabA_sh) if b < N_HB else \
                               (bhi, b - N_HB, tabB_sh)
                nc.sync.dma_start(bnc[bb * P:(bb + 1) * P, :], nm[:])
                if bb % CBLK == CBLK - 1:
                    pending.append((bnc, bb // CBLK, tab))

            def flush_ags(pending):
                for bnc, k, tab in pending:
                    nc.gpsimd.collective_compute(
                        "AllGather", mybir.AluOpType.bypass,
                        replica_groups=rg,
                        ins=[bnc[k * CROWS:(k + 1) * CROWS, :]],
                        outs=[tab[k * N_CORES * CROWS:
                                  (k + 1) * N_CORES * CROWS, :]])
                pending.clear()

            def propagate(tabA_, tabB_, out_fm, tx0_fm=None, table=None,
                          cheb=None):
                """out_fm = A_hat @ table (feature-major, per dst block).
                If tx0_fm: out = 2*prop - tx0 (second Chebyshev step).
                table=(blo,bhi,tabA,tabB): emit node-major table of out_fm,
                chunk-AllGathers flushed after each dst-half group. cheb:
                dict(tx0, tx1, wbase, bias_col, relu, h_fm, table, mlp)
                fused in pass B."""
                gmap = {}
                state = {"next_call": 0, "covered": 0}
                pending = []

                def ensure(tg):
                    while tg >= state["covered"]:
                        seg, t0, nt = calls[state["next_call"]]
                        g = gatp.tile([P, TILES_PER_CALL * P], F16, tag="g")
                        tab = tabA_ if seg % 2 == 0 else tabB_
                        col0 = (seg * sseg + t0 * P) // 16
                        nc.gpsimd.dma_gather(
                            out_ap=g[:, :nt * P]
                                .rearrange("p (n d) -> p n d", d=D),
                            in_ap=tab[:],
                            idxs_ap=idx_t[:, col0:col0 + nt * P // 16],
                            num_idxs=nt * P,
                            num_idxs_reg=nt * P,
                            elem_size=D,
                            queue_num=qctr[0] % N_QUEUES,
                            single_packet=False,
                        )
                        qctr[0] += 1
                        for k in range(nt):
                            gmap[seg * T_seg + t0 + k] = (g, k)
                        state["next_call"] += 1
                        state["covered"] = seg * T_seg + t0 + nt

                for dh in range(2):
                    for h in range(2):
                        seg = dh * 2 + h
                        for bb in range(N_HB):
                            b = dh * N_HB + bb
                            u = seg * N_HB + bb      # (seg, block) unit
                            s = selp.tile([P, sw], F16, tag="s")
                            seng = nc.scalar if sel_q[0] % 2 == 0 else nc.sync
                            sel_q[0] += 1
                            seng.dma_start(
                                s[:], smat_d[:, u * sw:(u + 1) * sw])
                            ps = psp.tile([P, P], F32, tag="ps", space="PSUM")
                            for k in range(t_half):
                                tg = seg * T_seg + bb * t_half + k
                                ensure(tg)
                                g, off = gmap[tg]
                                nc.tensor.matmul(
                                    out=ps[:],
                                    lhsT=g[:, off * P:(off + 1) * P],
                                    rhs=s[:, k * P:(k + 1) * P],
                                    start=(k == 0), stop=(k == t_half - 1),
                                )
                            blk = slice(b * P, (b + 1) * P)
                            pblk = slice(bb * P, (bb + 1) * P)
                            if h == 0:
                                nc.vector.tensor_copy(out=prt[:, pblk],
                                                      in_=ps[:])
                                continue
                            # ---- pass B: finalize block b -----------------
                            if tx0_fm is None:
                                nc.vector.tensor_tensor(
                                    out=out_fm[:, blk], in0=ps[:],
                                    in1=prt[:, pblk], op=mybir.AluOpType.add)
                            else:
                                t1 = tmpp.tile([P, P], F32, tag="t1")
                                nc.vector.scalar_tensor_tensor(
                                    out=t1[:], in0=prt[:, pblk], scalar=2.0,
                                    in1=tx0_fm[:, blk],
                                    op0=mybir.AluOpType.mult,
                                    op1=mybir.AluOpType.subtract)
                                nc.vector.scalar_tensor_tensor(
                                    out=out_fm[:, blk], in0=ps[:], scalar=2.0,
                                    in1=t1[:],
                                    op0=mybir.AluOpType.mult,
                                    op1=mybir.AluOpType.add)
                            if table is not None:
                                build_block_table(out_fm, blk, b, table,
                                                  pending)
                            if cheb is not None:
                                po = psop.tile([P, P], F32, tag="po",
                                               space="PSUM")
                                txs = (cheb["tx0"], cheb["tx1"], out_fm)
                                for k2, txk in enumerate(txs):
                                    nc.tensor.matmul(
                                        out=po[:],
                                        lhsT=wslice(cheb["wbase"] + k2),
                                        rhs=txk[:, blk],
                                        start=(k2 == 0), stop=(k2 == 2))
                                h_fm = cheb["h_fm"]
                                bc = cheb["bias_col"]
                                if cheb["relu"]:
                                    nc.scalar.activation(
                                        h_fm[:, blk], po[:],
                                        mybir.ActivationFunctionType.Relu,
                                        bias=bias_t[:, bc:bc + 1], scale=1.0)
                                else:
                                    nc.vector.tensor_scalar(
                                        out=h_fm[:, blk], in0=po[:],
                                        scalar1=bias_t[:, bc:bc + 1],
                                        scalar2=None,
                                        op0=mybir.AluOpType.add)
                                if cheb.get("table") is not None:
                                    build_block_table(h_fm, blk, b,
                                                      cheb["table"], pending)
                                if cheb.get("mlp"):
                                    pm = psop.tile([P, P], F32, tag="po",
                                                   space="PSUM")
                                    nc.tensor.matmul(out=pm[:],
                                                     lhsT=wslice(9),
                                                     rhs=h_fm[:, blk],
                                                     start=True, stop=True)
                                    h4 = nmp.tile([P, P], F16, tag="h4")
                                    nc.scalar.activation(
                                        h4[:], pm[:],
                                        mybir.ActivationFunctionType.Relu,
                                        bias=bias_t[:, 3:4], scale=1.0)
                                    p2 = psop.tile([1, P], F32, tag="po",
                                                   space="PSUM")
                                    nc.tensor.matmul(out=p2[:], lhsT=w2_ap,
                                                     rhs=h4[:],
                                                     start=True, stop=True)
                                    yo = nmp.tile([1, P], F32, tag="yo")
                                    nc.scalar.activation(
                                        yo[:], p2[:],
                                        mybir.ActivationFunctionType.Sigmoid,
                                        bias=b2_val, scale=1.0)
                                    nc.sync.dma_start(y_d[:, blk], yo[:1, :])
                    # flush this dst-half group's chunk AllGathers outside
                    # the gather stream (keeps Pool's SWDGE pipeline free)
                    if h == 1:
                        flush_ags(pending)

            # ================= layer 1 =================
            propagate(xA, xB, tB,
                      table=(bncA[0], bncB[0], tabsA[0], tabsB[0]))  # Tx1
            propagate(tabsA[0], tabsB[0], tC, tx0_fm=tA,
                      cheb=dict(tx0=tA, tx1=tB, wbase=0, bias_col=0,
                                relu=True, h_fm=tD,
                                table=(bncA[1], bncB[1], tabsA[1], tabsB[1])))

            # ================= layer 2 =================
            propagate(tabsA[1], tabsB[1], tB,
                      table=(bncA[2], bncB[2], tabsA[2], tabsB[2]))
            propagate(tabsA[2], tabsB[2], tC, tx0_fm=tD,
                      cheb=dict(tx0=tD, tx1=tB, wbase=3, bias_col=1,
                                relu=True, h_fm=tA,
                                table=(bncA[3], bncB[3], tabsA[3], tabsB[3])))

            # ================= layer 3 + MLP head =================
            propagate(tabsA[3], tabsB[3], tB,
                      table=(bncA[4], bncB[4], tabsA[4], tabsB[4]))
            propagate(tabsA[4], tabsB[4], tC, tx0_fm=tA,
                      cheb=dict(tx0=tA, tx1=tB, wbase=6, bias_col=2,
                                relu=False, h_fm=tD, mlp=True))

    nc.finalize()
    return nc


_CACHE = {}


def kernel(x, edge_index, edge_weight, W_in, b_in, W_hid, b_hid, W_out, b_out,
           mlp_w1, mlp_b1, bn_gamma, bn_beta, bn_mean, bn_var, mlp_w2, mlp_b2,
           _trace=False):
    x = np.asarray(x, dtype=np.float32)
    t_half, idx_all, smat_all = _preprocess(
        np.asarray(edge_index), np.asarray(edge_weight))

    b2_val = float(np.asarray(mlp_b2, np.float32).reshape(-1)[0])
    cache_key = (t_half, b2_val)
    if cache_key in _CACHE:
        nc = _CACHE[cache_key]
    else:
        nc = _build_program(t_half, b2_val)
        _CACHE[cache_key] = nc

    # ---- host-side tensor prep ----------------------------------------
    xpad = np.zeros((N_PAD, D), dtype=np.float32)
    xpad[:N_NODES] = x
    x16_np = xpad.astype(npf16)
    xA_t = _chunk_perm_half(x16_np[:HALF])
    xB_t = _chunk_perm_half(x16_np[HALF:])

    # BN folding: y = s*(h@W1 + b1) + t -> W1' = W1*s, b1' = b1*s + t
    s = (np.asarray(bn_gamma, np.float32)
         / np.sqrt(np.asarray(bn_var, np.float32) + BN_EPS))
    t_ = np.asarray(bn_beta, np.float32) - np.asarray(bn_mean, np.float32) * s
    w1p = np.asarray(mlp_w1, np.float32) * s[None, :]
    b1p = np.asarray(mlp_b1, np.float32) * s + t_

    wts = np.zeros((P, 10 * D + 1), dtype=npf16)
    for i, W in enumerate((W_in, W_hid, W_out)):
        W = np.asarray(W, np.float32)
        for k in range(K):
            wts[:, (i * K + k) * D:(i * K + k + 1) * D] = W[k].astype(npf16)
    wts[:, 9 * D:10 * D] = w1p.astype(npf16)
    wts[:, 10 * D:10 * D + 1] = np.asarray(mlp_w2, np.float32).astype(npf16)

    biases = np.zeros((P, 4), dtype=np.float32)
    biases[:, 0] = np.asarray(b_in, np.float32)
    biases[:, 1] = np.asarray(b_hid, np.float32)
    biases[:, 2] = np.asarray(b_out, np.float32)
    biases[:, 3] = b1p

    in_maps = []
    for c in range(N_CORES):
        own = np.concatenate([
            x16_np[c * HSLAB:(c + 1) * HSLAB],
            x16_np[HALF + c * HSLAB:HALF + (c + 1) * HSLAB],
        ], axis=0)  # [6400, 128]
        in_maps.append({
            "xA": xA_t,
            "xB": xB_t,
            "x0fm": np.ascontiguousarray(own.T),
            "idx": idx_all[c],
            "smat": smat_all[c],
            "wts": wts,
            "bias": biases,
        })

    res = run_bass_kernel_spmd(nc, in_maps, list(range(N_CORES)), trace=_trace)
    y_full = np.zeros(N_PAD, dtype=np.float32)
    for c in range(N_CORES):
        yc = res.results[c]["y"][0]
        y_full[c * HSLAB:(c + 1) * HSLAB] = yc[:HSLAB]
        y_full[HALF + c * HSLAB:HALF + (c + 1) * HSLAB] = yc[HSLAB:]
    out = y_full[:N_NODES, None].astype(np.float32)
    if _trace:
        kernel._last_results = res
    return out


# revision 16
# speedup vs baseline: 1.9341x; 1.8885x over previous
"""DeepChebNet (3-layer ChebConv K=3 + MLP head) on 8 Trainium2 NeuronCores.

Strategy (1D node partition per the sharding hint):
  - 50000 nodes padded to 51200, split into two 25600-row half-tables
    (int16 gather index limit). Each core owns 3200 nodes of each half
    (6400 total = 50 x 128-node dst blocks). Tables are laid out
    chunk-major (5 chunks x 8 cores x 640 rows per half) so each
    half-table AllGather splits into 5 contiguous chunk-AllGathers.
  - Each propagate processes dst-half group G1 (blocks 0..24) fully,
    then G2 (blocks 25..49); within a group, pass A (src half lo) then
    pass B (src half hi). The lo-table chunk-AllGathers are emitted
    right after G1 (and hi after G2) so they trigger at ~55% / 100% of
    the propagate and their transfers pipeline behind compute; keeping
    them out of the gather stream means the Pool engine's SWDGE
    pipeline is never clamped to the compute frontier mid-propagate.
  - Gathers are SWDGE dma_gather calls of 16 tiles (2048 descs)
    rotating across all 4 queues with a 4096-desc ring
    (dynamic_dma_scratch_size=65536): ~2.3 ns/desc aggregate vs ~8
    ns/desc when a call overflows the default 1024-desc ring, with the
    1.5us/call fixed desc-gen overhead amortized over 2048 descs.
  - Per-(src half, block) selection matrices (norm scattered by
    dst_local, fp16) are precomputed on host and streamed from DRAM on
    both HWDGE queues (sync/scalar alternating); PE accumulates
    G^T @ S into PSUM (feature-major out).
  - Pass A parks each block's PSUM in an fp16 SBUF partial; pass B adds
    its PSUM, applies the Chebyshev recurrence, and (fused, per block)
    the ChebConv output matmuls + bias/ReLU, the PE-transpose table
    rebuild, and on the last layer the MLP head + sigmoid.
"""
import numpy as np

import concourse.bacc as bacc
import concourse.bass as bass
import concourse.mybir as mybir
import concourse.tile as tile
from concourse.bass_utils import run_bass_kernel_spmd
from concourse.masks import make_identity

# problem constants (hardcoded per harness contract)
N_NODES = 50000
N_EDGES = 800000
D = 128
K = 3
BN_EPS = 1e-5

N_CORES = 8
P = 128
N_PAD = 51200
HALF = 25600            # rows per half-table (< 32768: int16-indexable)
HSLAB = 3200            # per-core nodes per half
BLK_NODES = 6400        # per-core nodes
N_BLOCKS = 50           # per-core 128-node dst blocks
N_HB = 25               # dst blocks per half per core
N_CHUNKS = 5            # AllGather chunks per half-table
CBLK = N_HB // N_CHUNKS  # dst blocks per AG chunk (5)
CROWS = CBLK * P         # rows per core per chunk (640)
TILES_PER_CALL = 8      # 1024 descs/call (ucode row cap), 4 per 4096 ring
N_QUEUES = 4
SCRATCH = 65536         # SWDGE ring: 4096 descs/queue

F16 = mybir.dt.float16
F32 = mybir.dt.float32
npf16 = np.float16


def _owner_block(n):
    """global node id -> (core, block 0..49) under the lo/hi layout."""
    lo = n < HALF
    core = np.where(lo, n // HSLAB, (n - HALF) // HSLAB)
    blk = np.where(lo, (n % HSLAB) // P, N_HB + ((n - HALF) % HSLAB) // P)
    return core, blk


def _table_row(pos):
    """position within a half (core-major) -> chunk-major table row."""
    c, sp = pos // HSLAB, pos % HSLAB
    return (sp // CROWS) * (N_CORES * CROWS) + c * CROWS + sp % CROWS


def _chunk_perm_half(xh):
    """[HALF, D] core-major half -> chunk-major table layout."""
    return np.ascontiguousarray(
        xh.reshape(N_CORES, N_CHUNKS, CROWS, D)
          .transpose(1, 0, 2, 3).reshape(HALF, D))


RHALF = 25000    # real nodes per half under the balanced split


def _balance(src, dst):
    """Balanced node->position permutation: per dst half, assign nodes to
    the 200 (core, block) bins greedily so every (bin, src-half) group has
    <= 1024 in-edges -> t_half drops from 9 to 8 (-11%% gather descs)."""
    hs = (src >= RHALF).astype(np.int64)
    ind_lo = np.bincount(dst[hs == 0], minlength=N_PAD)
    ind_hi = np.bincount(dst[hs == 1], minlength=N_PAD)
    pos = np.empty(N_PAD, dtype=np.int64)
    nbins = N_CORES * N_HB
    for dh in (0, 1):
        nodes = np.concatenate([
            np.arange(dh * RHALF, (dh + 1) * RHALF),
            np.arange(N_NODES + dh * (HALF - RHALF),
                      N_NODES + (dh + 1) * (HALF - RHALF))])
        dl = ind_lo[nodes].astype(np.int64)
        dhh = ind_hi[nodes].astype(np.int64)
        order = np.argsort(-(dl + dhh), kind="stable")
        sum_lo = np.zeros(nbins)
        sum_hi = np.zeros(nbins)
        cnt = np.zeros(nbins, np.int64)
        for i in order:
            cost = np.maximum(sum_lo + dl[i], sum_hi + dhh[i]) \
                + (cnt >= P) * 1e9
            b = int(np.argmin(cost))
            pos[nodes[i]] = dh * HALF + b * P + cnt[b]
            sum_lo[b] += dl[i]
            sum_hi[b] += dhh[i]
            cnt[b] += 1
    # bin index b maps to core b//N_HB, block b%N_HB under the core-major
    # slab layout: position dh*HALF + b*128 + s lands in core (b*128+s)
    # // HSLAB -- reorder so each bin is 128 consecutive rows of one core.
    return pos


def _preprocess(edge_index, edge_weight):
    """Graph partition -> per-core gather idx stream + selection matrices."""
    src = np.asarray(edge_index[0], dtype=np.int64)
    dst = np.asarray(edge_index[1], dtype=np.int64)
    w = np.asarray(edge_weight, dtype=np.float32)

    deg = np.bincount(src, weights=w.astype(np.float64), minlength=N_NODES)
    deg = deg.astype(np.float32)
    degs = np.sqrt(np.maximum(deg, 1e-38))
    dinv = np.where(deg > 0, 1.0 / degs, 0.0).astype(np.float32)
    norm = (-dinv[src] * w * dinv[dst]).astype(np.float32)

    pos = _balance(src, dst)
    src = pos[src]
    dst = pos[dst]

    core, blk = _owner_block(dst)
    half = (src >= HALF).astype(np.int64)
    dh = (blk >= N_HB).astype(np.int64)
    # stream order per core: dst-half group, then src half, then block
    key = (((core * 2 + dh) * 2 + half) * N_HB) + (blk - dh * N_HB)
    order = np.argsort(key, kind="stable")
    src_s, dst_s, norm_s, key_s = src[order], dst[order], norm[order], key[order]
    # chunk-major gather index within the src half-table
    srcl_s = _table_row(src_s - half[order] * HALF).astype(np.int16)
    dstl_s = (dst_s % P).astype(np.int64)

    n_groups = N_CORES * 4 * N_HB    # 800 (core, dh, half, block) groups
    bounds = np.searchsorted(key_s, np.arange(n_groups + 1))
    counts = bounds[1:] - bounds[:-1]
    # groups of pad dst nodes (>= N_NODES) are empty; their tiles are all-pad
    t_half = max(1, int(np.max((counts + P - 1) // P)))  # tiles per group

    T_seg = N_HB * t_half          # tiles per (dh, half) segment
    T_tot = 4 * T_seg
    sseg = T_seg * P

    idx_all, smat_all = [], []
    for c in range(N_CORES):
        e_src = np.zeros(4 * sseg, dtype=np.int16)
        e_dstl = np.zeros(4 * sseg, dtype=np.int64)
        e_norm = np.zeros(4 * sseg, dtype=np.float32)
        e_live = np.zeros(4 * sseg, dtype=bool)
        for seg in range(4):           # (dh, half)
            for b in range(N_HB):
                g = (c * 4 + seg) * N_HB + b
                lo, hi = bounds[g], bounds[g + 1]
                n = hi - lo
                base = (seg * N_HB + b) * t_half * P
                e_src[base:base + n] = srcl_s[lo:hi]
                e_dstl[base:base + n] = dstl_s[lo:hi]
                e_norm[base:base + n] = norm_s[lo:hi]
                e_live[base:base + n] = True
        # idx stream: per gather call, flat slot i -> (row i%16, col i//16),
        # replicated across the 8 groups of 16 partitions.
        idx16 = np.zeros((P, 4 * sseg // 16), dtype=np.int16)
        for seg in range(4):
            t0 = 0
            while t0 < T_seg:
                nt = min(TILES_PER_CALL, T_seg - t0)
                s0 = seg * sseg + t0 * P
                s = e_src[s0:s0 + nt * P]
                arr = s.reshape(nt * P // 16, 16).T      # [16, ncols]
                for gs in range(8):
                    idx16[gs * 16:(gs + 1) * 16,
                          s0 // 16:s0 // 16 + nt * P // 16] = arr
                t0 += nt
        idx_all.append(np.ascontiguousarray(idx16))
        # selection matrices: smat[p, T*P + j] = norm_e if slot (tile T,
        # lane p) holds edge e with dst_local j
        slot = np.nonzero(e_live)[0]
        s_all = np.zeros(4 * T_seg * P * P, dtype=npf16)
        s_all[slot * P + e_dstl[slot]] = e_norm[slot]
        smat = np.ascontiguousarray(
            s_all.reshape(4 * T_seg, P, P).transpose(1, 0, 2).reshape(P, -1))
        smat_all.append(smat)
    return t_half, idx_all, smat_all, pos


def _build_program(t_half, b2_val):
    """Build the SPMD Bass program (identical across cores)."""
    nc = bacc.Bacc("TRN2", target_bir_lowering=False, debug=False,
                   num_devices=N_CORES, num_swdge_queues=N_QUEUES,
                   dynamic_dma_scratch_size=SCRATCH)

    T_seg = N_HB * t_half
    sseg = T_seg * P
    sw = t_half * P                 # smat cols per (seg, block) unit
    calls = []                      # (seg, t0, nt) in stream order
    for seg in range(4):
        t0 = 0
        while t0 < T_seg:
            nt = min(TILES_PER_CALL, T_seg - t0)
            calls.append((seg, t0, nt))
            t0 += nt

    # ---- I/O -----------------------------------------------------------
    xA = nc.dram_tensor("xA", [HALF, D], F16, kind="ExternalInput")
    xB = nc.dram_tensor("xB", [HALF, D], F16, kind="ExternalInput")
    x0fm = nc.dram_tensor("x0fm", [P, BLK_NODES], F16, kind="ExternalInput")
    idx_d = nc.dram_tensor("idx", [P, 4 * sseg // 16], mybir.dt.int16,
                           kind="ExternalInput")
    smat_d = nc.dram_tensor("smat", [P, 4 * T_seg * P], F16,
                            kind="ExternalInput")
    wts_d = nc.dram_tensor("wts", [P, 10 * D + 1], F16, kind="ExternalInput")
    bias_d = nc.dram_tensor("bias", [P, 4], F32, kind="ExternalInput")
    y_d = nc.dram_tensor("y", [1, BLK_NODES], F32, kind="ExternalOutput")

    tabsA = [nc.dram_tensor(f"tabA{i}", [HALF, D], F16, addr_space="Shared")
             for i in range(5)]
    tabsB = [nc.dram_tensor(f"tabB{i}", [HALF, D], F16, addr_space="Shared")
             for i in range(5)]
    rg = [list(range(N_CORES))]

    with tile.TileContext(nc) as tc:
        with (
            tc.tile_pool(name="const", bufs=1) as constp,
            tc.tile_pool(name="big", bufs=1) as bigp,
            tc.tile_pool(name="gat", bufs=20) as gatp,
            tc.tile_pool(name="sel", bufs=4) as selp,
            tc.tile_pool(name="nm", bufs=3) as nmp,
            tc.tile_pool(name="tmp", bufs=2) as tmpp,
            tc.tile_pool(name="ps", bufs=4, space="PSUM") as psp,
            tc.tile_pool(name="pst", bufs=2, space="PSUM") as pstp,
            tc.tile_pool(name="pso", bufs=2, space="PSUM") as psop,
            tc.tile_pool(name="dram", bufs=1, space="DRAM") as dramp,
        ):
            # ---- load constants -----------------------------------------
            idx_t = constp.tile([P, 4 * sseg // 16], mybir.dt.int16)
            wts_t = constp.tile([P, 10 * D + 1], F16)
            bias_t = constp.tile([P, 4], F32)
            ident = constp.tile([P, P], F16)
            nc.sync.dma_start(idx_t[:], idx_d[:])
            nc.sync.dma_start(wts_t[:], wts_d[:])
            nc.sync.dma_start(bias_t[:], bias_d[:])
            make_identity(nc, ident[:])

            def wslice(i):  # i-th [P, D] weight block (lhsT layout [fi, fo])
                return wts_t[:, i * D:(i + 1) * D]

            w2_ap = wts_t[:, 10 * D:10 * D + 1]

            # ---- feature-major activations [P, 6400] f16 + f16 partial --
            tA = bigp.tile([P, BLK_NODES], F16, tag="tA")
            tB = bigp.tile([P, BLK_NODES], F16, tag="tB")
            tC = bigp.tile([P, BLK_NODES], F16, tag="tC")
            tD = bigp.tile([P, BLK_NODES], F16, tag="tD")
            prt = bigp.tile([P, N_HB * P], F16, tag="prt")
            nc.sync.dma_start(tA[:], x0fm[:])

            bncA = [dramp.tile([HSLAB, D], F16, tag=f"bncA{i}", name=f"bncA{i}")
                    for i in range(5)]
            bncB = [dramp.tile([HSLAB, D], F16, tag=f"bncB{i}", name=f"bncB{i}")
                    for i in range(5)]

            qctr = [0]
            sel_q = [0]

            def build_block_table(src_fm, blk, b, table, pending):
                """PE-transpose one fm block into the node-major DRAM slab;
                queue a chunk AllGather when its 5th block lands."""
                blo, bhi, tabA_sh, tabB_sh = table
                pt = pstp.tile([P, P], F16, tag="pt", space="PSUM")
                nc.tensor.transpose(pt[:], src_fm[:, blk], ident[:])
                nm = nmp.tile([P, P], F16, tag="nm")
                nc.scalar.activation(nm[:], pt[:],
                                     mybir.ActivationFunctionType.Copy,
                                     scale=1.0)
                bnc, bb, tab = (blo, b, tabA_sh) if b < N_HB else \
                               (bhi, b - N_HB, tabB_sh)
                nc.sync.dma_start(bnc[bb * P:(bb + 1) * P, :], nm[:])
                if bb % CBLK == CBLK - 1:
                    pending.append((bnc, bb // CBLK, tab))

            def flush_ags(pending):
                for bnc, k, tab in pending:
                    nc.gpsimd.collective_compute(
                        "AllGather", mybir.AluOpType.bypass,
                        replica_groups=rg,
                        ins=[bnc[k * CROWS:(k + 1) * CROWS, :]],
                        outs=[tab[k * N_CORES * CROWS:
                                  (k + 1) * N_CORES * CROWS, :]])
                pending.clear()

            def propagate(tabA_, tabB_, out_fm, tx0_fm=None, table=None,
                          cheb=None):
                """out_fm = A_hat @ table (feature-major, per dst block).
                If tx0_fm: out = 2*prop - tx0 (second Chebyshev step).
                table=(blo,bhi,tabA,tabB): emit node-major table of out_fm,
                chunk-AllGathers flushed after each dst-half group. cheb:
                dict(tx0, tx1, wbase, bias_col, relu, h_fm, table, mlp)
                fused in pass B."""
                gmap = {}
                state = {"next_call": 0, "covered": 0}
                pending = []

                def ensure(tg):
                    while tg >= state["covered"]:
                        seg, t0, nt = calls[state["next_call"]]
                        g = gatp.tile([P, TILES_PER_CALL * P], F16, tag="g")
                        tab = tabA_ if seg % 2 == 0 else tabB_
                        col0 = (seg * sseg + t0 * P) // 16
                        nc.gpsimd.dma_gather(
                            out_ap=g[:, :nt * P]
                                .rearrange("p (n d) -> p n d", d=D),
                            in_ap=tab[:],
                            idxs_ap=idx_t[:, col0:col0 + nt * P // 16],
                            num_idxs=nt * P,
                            num_idxs_reg=nt * P,
                            elem_size=D,
                            queue_num=qctr[0] % N_QUEUES,
                            single_packet=False,
                        )
                        qctr[0] += 1
                        for k in range(nt):
                            gmap[seg * T_seg + t0 + k] = (g, k)
                        state["next_call"] += 1
                        state["covered"] = seg * T_seg + t0 + nt

                for dh in range(2):
                    for h in range(2):
                        seg = dh * 2 + h
                        for bb in range(N_HB):
                            b = dh * N_HB + bb
                            u = seg * N_HB + bb      # (seg, block) unit
                            s = selp.tile([P, sw], F16, tag="s")
                            seng = nc.scalar if sel_q[0] % 2 == 0 else nc.sync
                            sel_q[0] += 1
                            seng.dma_start(
                                s[:], smat_d[:, u * sw:(u + 1) * sw])
                            ps = psp.tile([P, P], F32, tag="ps", space="PSUM")
                            for k in range(t_half):
                                tg = seg * T_seg + bb * t_half + k
                                ensure(tg)
                                g, off = gmap[tg]
                                nc.tensor.matmul(
                                    out=ps[:],
                                    lhsT=g[:, off * P:(off + 1) * P],
                                    rhs=s[:, k * P:(k + 1) * P],
                                    start=(k == 0), stop=(k == t_half - 1),
                                )
                            blk = slice(b * P, (b + 1) * P)
                            pblk = slice(bb * P, (bb + 1) * P)
                            if h == 0:
                                nc.vector.tensor_copy(out=prt[:, pblk],
                                                      in_=ps[:])
                                continue
                            # ---- pass B: finalize block b -----------------
                            if tx0_fm is None:
                                nc.vector.tensor_tensor(
                                    out=out_fm[:, blk], in0=ps[:],
                                    in1=prt[:, pblk], op=mybir.AluOpType.add)
                            else:
                                t1 = tmpp.tile([P, P], F32, tag="t1")
                                nc.vector.scalar_tensor_tensor(
                                    out=t1[:], in0=prt[:, pblk], scalar=2.0,
                                    in1=tx0_fm[:, blk],
                                    op0=mybir.AluOpType.mult,
                                    op1=mybir.AluOpType.subtract)
                                nc.vector.scalar_tensor_tensor(
                                    out=out_fm[:, blk], in0=ps[:], scalar=2.0,
                                    in1=t1[:],
                                    op0=mybir.AluOpType.mult,
                                    op1=mybir.AluOpType.add)
                            if table is not None:
                                build_block_table(out_fm, blk, b, table,
                                                  pending)
                            if cheb is not None:
                                po = psop.tile([P, P], F32, tag="po",
                                               space="PSUM")
                                txs = (cheb["tx0"], cheb["tx1"], out_fm)
                                for k2, txk in enumerate(txs):
                                    nc.tensor.matmul(
                                        out=po[:],
                                        lhsT=wslice(cheb["wbase"] + k2),
                                        rhs=txk[:, blk],
                                        start=(k2 == 0), stop=(k2 == 2))
                                h_fm = cheb["h_fm"]
                                bc = cheb["bias_col"]
                                if cheb["relu"]:
                                    nc.scalar.activation(
                                        h_fm[:, blk], po[:],
                                        mybir.ActivationFunctionType.Relu,
                                        bias=bias_t[:, bc:bc + 1], scale=1.0)
                                else:
                                    nc.vector.tensor_scalar(
                                        out=h_fm[:, blk], in0=po[:],
                                        scalar1=bias_t[:, bc:bc + 1],
                                        scalar2=None,
                                        op0=mybir.AluOpType.add)
                                if cheb.get("table") is not None:
                                    build_block_table(h_fm, blk, b,
                                                      cheb["table"], pending)
                                if cheb.get("mlp"):
                                    pm = psop.tile([P, P], F32, tag="po",
                                                   space="PSUM")
                                    nc.tensor.matmul(out=pm[:],
                                                     lhsT=wslice(9),
                                                     rhs=h_fm[:, blk],
                                                     start=True, stop=True)
                                    h4 = nmp.tile([P, P], F16, tag="h4")
                                    nc.scalar.activation(
                                        h4[:], pm[:],
                                        mybir.ActivationFunctionType.Relu,
                                        bias=bias_t[:, 3:4], scale=1.0)
                                    p2 = psop.tile([1, P], F32, tag="po",
                                                   space="PSUM")
                                    nc.tensor.matmul(out=p2[:], lhsT=w2_ap,
                                                     rhs=h4[:],
                                                     start=True, stop=True)
                                    yo = nmp.tile([1, P], F32, tag="yo")
                                    nc.scalar.activation(
                                        yo[:], p2[:],
                                        mybir.ActivationFunctionType.Sigmoid,
                                        bias=b2_val, scale=1.0)
                                    nc.sync.dma_start(y_d[:, blk], yo[:1, :])
                        # flush G1's lo-table AllGathers after G2's pass-A
                        # gather emission so Pool never stalls on them while
                        # gather work remains; G2's at the propagate end.
                        if (dh, h) == (1, 0) or (dh, h) == (1, 1):
                            flush_ags(pending)

            # ================= layer 1 =================
            propagate(xA, xB, tB,
                      table=(bncA[0], bncB[0], tabsA[0], tabsB[0]))  # Tx1
            propagate(tabsA[0], tabsB[0], tC, tx0_fm=tA,
                      cheb=dict(tx0=tA, tx1=tB, wbase=0, bias_col=0,
                                relu=True, h_fm=tD,
                                table=(bncA[1], bncB[1], tabsA[1], tabsB[1])))

            # ================= layer 2 =================
            propagate(tabsA[1], tabsB[1], tB,
                      table=(bncA[2], bncB[2], tabsA[2], tabsB[2]))
            propagate(tabsA[2], tabsB[2], tC, tx0_fm=tD,
                      cheb=dict(tx0=tD, tx1=tB, wbase=3, bias_col=1,
                                relu=True, h_fm=tA,
                                table=(bncA[3], bncB[3], tabsA[3], tabsB[3])))

            # ================= layer 3 + MLP head =================
            propagate(tabsA[3], tabsB[3], tB,
                      table=(bncA[4], bncB[4], tabsA[4], tabsB[4]))
            propagate(tabsA[4], tabsB[4], tC, tx0_fm=tA,
                      cheb=dict(tx0=tA, tx1=tB, wbase=6, bias_col=2,
                                relu=False, h_fm=tD, mlp=True))

    nc.finalize()
    return nc


_CACHE = {}


def kernel(x, edge_index, edge_weight, W_in, b_in, W_hid, b_hid, W_out, b_out,
           mlp_w1, mlp_b1, bn_gamma, bn_beta, bn_mean, bn_var, mlp_w2, mlp_b2,
           _trace=False):
    x = np.asarray(x, dtype=np.float32)
    t_half, idx_all, smat_all, pos = _preprocess(
        np.asarray(edge_index), np.asarray(edge_weight))

    b2_val = float(np.asarray(mlp_b2, np.float32).reshape(-1)[0])
    cache_key = (t_half, b2_val)
    if cache_key in _CACHE:
        nc = _CACHE[cache_key]
    else:
        nc = _build_program(t_half, b2_val)
        _CACHE[cache_key] = nc

    # ---- host-side tensor prep (x rows permuted to balanced positions) --
    xpad = np.zeros((N_PAD, D), dtype=np.float32)
    xpad[pos[:N_NODES]] = x
    x16_np = xpad.astype(npf16)
    xA_t = _chunk_perm_half(x16_np[:HALF])
    xB_t = _chunk_perm_half(x16_np[HALF:])

    # BN folding: y = s*(h@W1 + b1) + t -> W1' = W1*s, b1' = b1*s + t
    s = (np.asarray(bn_gamma, np.float32)
         / np.sqrt(np.asarray(bn_var, np.float32) + BN_EPS))
    t_ = np.asarray(bn_beta, np.float32) - np.asarray(bn_mean, np.float32) * s
    w1p = np.asarray(mlp_w1, np.float32) * s[None, :]
    b1p = np.asarray(mlp_b1, np.float32) * s + t_

    wts = np.zeros((P, 10 * D + 1), dtype=npf16)
    for i, W in enumerate((W_in, W_hid, W_out)):
        W = np.asarray(W, np.float32)
        for k in range(K):
            wts[:, (i * K + k) * D:(i * K + k + 1) * D] = W[k].astype(npf16)
    wts[:, 9 * D:10 * D] = w1p.astype(npf16)
    wts[:, 10 * D:10 * D + 1] = np.asarray(mlp_w2, np.float32).astype(npf16)

    biases = np.zeros((P, 4), dtype=np.float32)
    biases[:, 0] = np.asarray(b_in, np.float32)
    biases[:, 1] = np.asarray(b_hid, np.float32)
    biases[:, 2] = np.asarray(b_out, np.float32)
    biases[:, 3] = b1p

    in_maps = []
    for c in range(N_CORES):
        own = np.concatenate([
            x16_np[c * HSLAB:(c + 1) * HSLAB],
            x16_np[HALF + c * HSLAB:HALF + (c + 1) * HSLAB],
        ], axis=0)  # [6400, 128]
        in_maps.append({
            "xA": xA_t,
            "xB": xB_t,
            "x0fm": np.ascontiguousarray(own.T),
            "idx": idx_all[c],
            "smat": smat_all[c],
            "wts": wts,
            "bias": biases,
        })

    res = run_bass_kernel_spmd(nc, in_maps, list(range(N_CORES)), trace=_trace)
    y_full = np.zeros(N_PAD, dtype=np.float32)
    for c in range(N_CORES):
        yc = res.results[c]["y"][0]
        y_full[c * HSLAB:(c + 1) * HSLAB] = yc[:HSLAB]
        y_full[HALF + c * HSLAB:HALF + (c + 1) * HSLAB] = yc[HSLAB:]
    out = y_full[pos[:N_NODES], None].astype(np.float32)
    if _trace:
        kernel._last_results = res
    return out


# revision 19
# speedup vs baseline: 2.2975x; 1.1879x over previous
"""DeepChebNet (3-layer ChebConv K=3 + MLP head) on 8 Trainium2 NeuronCores.

Strategy (1D node partition per the sharding hint):
  - 50000 nodes padded to 51200, split into two 25600-row half-tables
    (int16 gather index limit). Each core owns 3200 nodes of each half
    (6400 total = 50 x 128-node dst blocks). Tables are laid out
    chunk-major (5 chunks x 8 cores x 640 rows per half) so each
    half-table AllGather splits into 5 contiguous chunk-AllGathers.
  - Each propagate processes dst-half group G1 (blocks 0..24) fully,
    then G2 (blocks 25..49); within a group, pass A (src half lo) then
    pass B (src half hi). The lo-table chunk-AllGathers are emitted
    right after G1 (and hi after G2) so they trigger at ~55% / 100% of
    the propagate and their transfers pipeline behind compute; keeping
    them out of the gather stream means the Pool engine's SWDGE
    pipeline is never clamped to the compute frontier mid-propagate.
  - Gathers are SWDGE dma_gather calls of 16 tiles (2048 descs)
    rotating across all 4 queues with a 4096-desc ring
    (dynamic_dma_scratch_size=65536): ~2.3 ns/desc aggregate vs ~8
    ns/desc when a call overflows the default 1024-desc ring, with the
    1.5us/call fixed desc-gen overhead amortized over 2048 descs.
  - Per-(src half, block) selection matrices (norm scattered by
    dst_local, fp16) are precomputed on host and streamed from DRAM on
    both HWDGE queues (sync/scalar alternating); PE accumulates
    G^T @ S into PSUM (feature-major out).
  - Pass A parks each block's PSUM in an fp16 SBUF partial; pass B adds
    its PSUM, applies the Chebyshev recurrence, and (fused, per block)
    the ChebConv output matmuls + bias/ReLU, the PE-transpose table
    rebuild, and on the last layer the MLP head + sigmoid.
"""
import numpy as np

import concourse.bacc as bacc
import concourse.bass as bass
import concourse.mybir as mybir
import concourse.tile as tile
from concourse.bass_utils import run_bass_kernel_spmd
from concourse.masks import make_identity

# problem constants (hardcoded per harness contract)
N_NODES = 50000
N_EDGES = 800000
D = 128
K = 3
BN_EPS = 1e-5

N_CORES = 8
P = 128
N_PAD = 51200
HALF = 25600            # rows per half-table (< 32768: int16-indexable)
HSLAB = 3200            # per-core nodes per half
BLK_NODES = 6400        # per-core nodes
N_BLOCKS = 50           # per-core 128-node dst blocks
N_HB = 25               # dst blocks per half per core
N_CHUNKS = 5            # AllGather chunks per half-table
CBLK = N_HB // N_CHUNKS  # dst blocks per AG chunk (5)
CROWS = CBLK * P         # rows per core per chunk (640)
TILES_PER_CALL = 8      # 1024 descs/call (ucode row cap), 4 per 4096 ring
N_QUEUES = 4
SCRATCH = 65536         # SWDGE ring: 4096 descs/queue

F16 = mybir.dt.float16
F32 = mybir.dt.float32
npf16 = np.float16


def _owner_block(n):
    """global node id -> (core, block 0..49) under the lo/hi layout."""
    lo = n < HALF
    core = np.where(lo, n // HSLAB, (n - HALF) // HSLAB)
    blk = np.where(lo, (n % HSLAB) // P, N_HB + ((n - HALF) % HSLAB) // P)
    return core, blk


def _table_row(pos):
    """position within a half (core-major) -> chunk-major table row."""
    c, sp = pos // HSLAB, pos % HSLAB
    return (sp // CROWS) * (N_CORES * CROWS) + c * CROWS + sp % CROWS


def _chunk_perm_half(xh):
    """[HALF, D] core-major half -> chunk-major table layout."""
    return np.ascontiguousarray(
        xh.reshape(N_CORES, N_CHUNKS, CROWS, D)
          .transpose(1, 0, 2, 3).reshape(HALF, D))


RHALF = 25000    # real nodes per half under the balanced split


def _balance(src, dst):
    """Balanced node->position permutation: per dst half, assign nodes to
    the 200 (core, block) bins greedily so every (bin, src-half) group has
    <= 1024 in-edges -> t_half drops from 9 to 8 (-11%% gather descs)."""
    hs = (src >= RHALF).astype(np.int64)
    ind_lo = np.bincount(dst[hs == 0], minlength=N_PAD)
    ind_hi = np.bincount(dst[hs == 1], minlength=N_PAD)
    pos = np.empty(N_PAD, dtype=np.int64)
    nbins = N_CORES * N_HB
    for dh in (0, 1):
        nodes = np.concatenate([
            np.arange(dh * RHALF, (dh + 1) * RHALF),
            np.arange(N_NODES + dh * (HALF - RHALF),
                      N_NODES + (dh + 1) * (HALF - RHALF))])
        dl = ind_lo[nodes].astype(np.int64)
        dhh = ind_hi[nodes].astype(np.int64)
        order = np.argsort(-(dl + dhh), kind="stable")
        sum_lo = np.zeros(nbins)
        sum_hi = np.zeros(nbins)
        cnt = np.zeros(nbins, np.int64)
        for i in order:
            cost = np.maximum(sum_lo + dl[i], sum_hi + dhh[i]) \
                + (cnt >= P) * 1e9
            b = int(np.argmin(cost))
            pos[nodes[i]] = dh * HALF + b * P + cnt[b]
            sum_lo[b] += dl[i]
            sum_hi[b] += dhh[i]
            cnt[b] += 1
    # bin index b maps to core b//N_HB, block b%N_HB under the core-major
    # slab layout: position dh*HALF + b*128 + s lands in core (b*128+s)
    # // HSLAB -- reorder so each bin is 128 consecutive rows of one core.
    return pos


def _preprocess(edge_index, edge_weight):
    """Graph partition -> per-core gather idx stream + selection matrices."""
    src = np.asarray(edge_index[0], dtype=np.int64)
    dst = np.asarray(edge_index[1], dtype=np.int64)
    w = np.asarray(edge_weight, dtype=np.float32)

    deg = np.bincount(src, weights=w.astype(np.float64), minlength=N_NODES)
    deg = deg.astype(np.float32)
    degs = np.sqrt(np.maximum(deg, 1e-38))
    dinv = np.where(deg > 0, 1.0 / degs, 0.0).astype(np.float32)
    norm = (-dinv[src] * w * dinv[dst]).astype(np.float32)

    pos = _balance(src, dst)
    src = pos[src]
    dst = pos[dst]

    core, blk = _owner_block(dst)
    half = (src >= HALF).astype(np.int64)
    dh = (blk >= N_HB).astype(np.int64)
    # stream order per core: dst-half group, then src half, then block
    key = (((core * 2 + dh) * 2 + half) * N_HB) + (blk - dh * N_HB)
    order = np.argsort(key, kind="stable")
    src_s, dst_s, norm_s, key_s = src[order], dst[order], norm[order], key[order]
    # chunk-major gather index within the src half-table
    srcl_s = _table_row(src_s - half[order] * HALF).astype(np.int16)
    dstl_s = (dst_s % P).astype(np.int64)

    n_groups = N_CORES * 4 * N_HB    # 800 (core, dh, half, block) groups
    bounds = np.searchsorted(key_s, np.arange(n_groups + 1))
    counts = bounds[1:] - bounds[:-1]
    # groups of pad dst nodes (>= N_NODES) are empty; their tiles are all-pad
    t_half = max(1, int(np.max((counts + P - 1) // P)))  # tiles per group

    T_seg = N_HB * t_half          # tiles per (dh, half) segment
    T_tot = 4 * T_seg
    sseg = T_seg * P

    idx_all, smat_all = [], []
    for c in range(N_CORES):
        e_src = np.zeros(4 * sseg, dtype=np.int16)
        e_dstl = np.zeros(4 * sseg, dtype=np.int64)
        e_norm = np.zeros(4 * sseg, dtype=np.float32)
        e_live = np.zeros(4 * sseg, dtype=bool)
        for seg in range(4):           # (dh, half)
            for b in range(N_HB):
                g = (c * 4 + seg) * N_HB + b
                lo, hi = bounds[g], bounds[g + 1]
                n = hi - lo
                base = (seg * N_HB + b) * t_half * P
                e_src[base:base + n] = srcl_s[lo:hi]
                e_dstl[base:base + n] = dstl_s[lo:hi]
                e_norm[base:base + n] = norm_s[lo:hi]
                e_live[base:base + n] = True
        # idx stream: per gather call, flat slot i -> (row i%16, col i//16),
        # replicated across the 8 groups of 16 partitions.
        idx16 = np.zeros((P, 4 * sseg // 16), dtype=np.int16)
        for seg in range(4):
            t0 = 0
            while t0 < T_seg:
                nt = min(TILES_PER_CALL, T_seg - t0)
                s0 = seg * sseg + t0 * P
                s = e_src[s0:s0 + nt * P]
                arr = s.reshape(nt * P // 16, 16).T      # [16, ncols]
                for gs in range(8):
                    idx16[gs * 16:(gs + 1) * 16,
                          s0 // 16:s0 // 16 + nt * P // 16] = arr
                t0 += nt
        idx_all.append(np.ascontiguousarray(idx16))
        # selection matrices: smat[p, T*P + j] = norm_e if slot (tile T,
        # lane p) holds edge e with dst_local j
        slot = np.nonzero(e_live)[0]
        s_all = np.zeros(4 * T_seg * P * P, dtype=npf16)
        s_all[slot * P + e_dstl[slot]] = e_norm[slot]
        smat = np.ascontiguousarray(
            s_all.reshape(4 * T_seg, P, P).transpose(1, 0, 2).reshape(P, -1))
        smat_all.append(smat)
    return t_half, idx_all, smat_all, pos


def _build_program(t_half, b2_val):
    """Build the SPMD Bass program (identical across cores)."""
    nc = bacc.Bacc("TRN2", target_bir_lowering=False, debug=False,
                   num_devices=N_CORES, num_swdge_queues=N_QUEUES,
                   dynamic_dma_scratch_size=SCRATCH)

    T_seg = N_HB * t_half
    sseg = T_seg * P
    sw = t_half * P                 # smat cols per (seg, block) unit
    calls = []                      # (seg, t0, nt) in stream order
    for seg in range(4):
        t0 = 0
        while t0 < T_seg:
            nt = min(TILES_PER_CALL, T_seg - t0)
            calls.append((seg, t0, nt))
            t0 += nt

    # ---- I/O -----------------------------------------------------------
    xA = nc.dram_tensor("xA", [HALF, D], F16, kind="ExternalInput")
    xB = nc.dram_tensor("xB", [HALF, D], F16, kind="ExternalInput")
    x0fm = nc.dram_tensor("x0fm", [P, BLK_NODES], F16, kind="ExternalInput")
    idx_d = nc.dram_tensor("idx", [P, 4 * sseg // 16], mybir.dt.int16,
                           kind="ExternalInput")
    smat_d = nc.dram_tensor("smat", [P, 4 * T_seg * P], F16,
                            kind="ExternalInput")
    wts_d = nc.dram_tensor("wts", [P, 10 * D + 1], F16, kind="ExternalInput")
    bias_d = nc.dram_tensor("bias", [P, 4], F32, kind="ExternalInput")
    y_d = nc.dram_tensor("y", [1, BLK_NODES], F32, kind="ExternalOutput")

    tabsA = [nc.dram_tensor(f"tabA{i}", [HALF, D], F16, addr_space="Shared")
             for i in range(5)]
    tabsB = [nc.dram_tensor(f"tabB{i}", [HALF, D], F16, addr_space="Shared")
             for i in range(5)]
    rg = [list(range(N_CORES))]

    with tile.TileContext(nc) as tc:
        with (
            tc.tile_pool(name="const", bufs=1) as constp,
            tc.tile_pool(name="big", bufs=1) as bigp,
            tc.tile_pool(name="gat", bufs=20) as gatp,
            tc.tile_pool(name="sel", bufs=4) as selp,
            tc.tile_pool(name="nm", bufs=3) as nmp,
            tc.tile_pool(name="tmp", bufs=2) as tmpp,
            tc.tile_pool(name="ps", bufs=4, space="PSUM") as psp,
            tc.tile_pool(name="pst", bufs=2, space="PSUM") as pstp,
            tc.tile_pool(name="pso", bufs=2, space="PSUM") as psop,
            tc.tile_pool(name="dram", bufs=1, space="DRAM") as dramp,
        ):
            # ---- load constants -----------------------------------------
            idx_t = constp.tile([P, 4 * sseg // 16], mybir.dt.int16)
            wts_t = constp.tile([P, 10 * D + 1], F16)
            bias_t = constp.tile([P, 4], F32)
            ident = constp.tile([P, P], F16)
            nc.sync.dma_start(idx_t[:], idx_d[:])
            nc.sync.dma_start(wts_t[:], wts_d[:])
            nc.sync.dma_start(bias_t[:], bias_d[:])
            make_identity(nc, ident[:])

            def wslice(i):  # i-th [P, D] weight block (lhsT layout [fi, fo])
                return wts_t[:, i * D:(i + 1) * D]

            w2_ap = wts_t[:, 10 * D:10 * D + 1]

            # ---- feature-major activations [P, 6400] f16 + f16 partial --
            tA = bigp.tile([P, BLK_NODES], F16, tag="tA")
            tB = bigp.tile([P, BLK_NODES], F16, tag="tB")
            tC = bigp.tile([P, BLK_NODES], F16, tag="tC")
            tD = bigp.tile([P, BLK_NODES], F16, tag="tD")
            prt = bigp.tile([P, N_HB * P], F16, tag="prt")
            nc.sync.dma_start(tA[:], x0fm[:])

            bncA = [dramp.tile([HSLAB, D], F16, tag=f"bncA{i}", name=f"bncA{i}")
                    for i in range(5)]
            bncB = [dramp.tile([HSLAB, D], F16, tag=f"bncB{i}", name=f"bncB{i}")
                    for i in range(5)]

            qctr = [0]
            sel_q = [0]

            def build_block_table(src_fm, blk, b, table, pending):
                """PE-transpose one fm block into the node-major DRAM slab;
                queue a chunk AllGather when its 5th block lands."""
                blo, bhi, tabA_sh, tabB_sh = table
                pt = pstp.tile([P, P], F16, tag="pt", space="PSUM")
                nc.tensor.transpose(pt[:], src_fm[:, blk], ident[:])
                nm = nmp.tile([P, P], F16, tag="nm")
                nc.scalar.activation(nm[:], pt[:],
                                     mybir.ActivationFunctionType.Copy,
                                     scale=1.0)
                bnc, bb, tab = (blo, b, tabA_sh) if b < N_HB else \
                               (bhi, b - N_HB, tabB_sh)
                nc.sync.dma_start(bnc[bb * P:(bb + 1) * P, :], nm[:])
                if bb % CBLK == CBLK - 1:
                    pending.append((bnc, bb // CBLK, tab))

            def flush_ags(pending):
                for bnc, k, tab in pending:
                    nc.gpsimd.collective_compute(
                        "AllGather", mybir.AluOpType.bypass,
                        replica_groups=rg,
                        ins=[bnc[k * CROWS:(k + 1) * CROWS, :]],
                        outs=[tab[k * N_CORES * CROWS:
                                  (k + 1) * N_CORES * CROWS, :]])
                pending.clear()

            def propagate(tabA_, tabB_, out_fm, tx0_fm=None, table=None,
                          cheb=None):
                """out_fm = A_hat @ table (feature-major, per dst block).
                If tx0_fm: out = 2*prop - tx0 (second Chebyshev step).
                table=(blo,bhi,tabA,tabB): emit node-major table of out_fm,
                chunk-AllGathers flushed after each dst-half group. cheb:
                dict(tx0, tx1, wbase, bias_col, relu, h_fm, table, mlp)
                fused in pass B."""
                gmap = {}
                state = {"next_call": 0, "covered": 0}
                pending = []

                def ensure(tg):
                    while tg >= state["covered"]:
                        seg, t0, nt = calls[state["next_call"]]
                        g = gatp.tile([P, TILES_PER_CALL * P], F16, tag="g")
                        tab = tabA_ if seg % 2 == 0 else tabB_
                        col0 = (seg * sseg + t0 * P) // 16
                        nc.gpsimd.dma_gather(
                            out_ap=g[:, :nt * P]
                                .rearrange("p (n d) -> p n d", d=D),
                            in_ap=tab[:],
                            idxs_ap=idx_t[:, col0:col0 + nt * P // 16],
                            num_idxs=nt * P,
                            num_idxs_reg=nt * P,
                            elem_size=D,
                            queue_num=qctr[0] % N_QUEUES,
                            single_packet=True,
                        )
                        qctr[0] += 1
                        for k in range(nt):
                            gmap[seg * T_seg + t0 + k] = (g, k)
                        state["next_call"] += 1
                        state["covered"] = seg * T_seg + t0 + nt

                for dh in range(2):
                    for h in range(2):
                        seg = dh * 2 + h
                        for bb in range(N_HB):
                            b = dh * N_HB + bb
                            u = seg * N_HB + bb      # (seg, block) unit
                            s = selp.tile([P, sw], F16, tag="s")
                            seng = nc.scalar if sel_q[0] % 2 == 0 else nc.sync
                            sel_q[0] += 1
                            seng.dma_start(
                                s[:], smat_d[:, u * sw:(u + 1) * sw])
                            ps = psp.tile([P, P], F32, tag="ps", space="PSUM")
                            for k in range(t_half):
                                tg = seg * T_seg + bb * t_half + k
                                ensure(tg)
                                g, off = gmap[tg]
                                nc.tensor.matmul(
                                    out=ps[:],
                                    lhsT=g[:, off * P:(off + 1) * P],
                                    rhs=s[:, k * P:(k + 1) * P],
                                    start=(k == 0), stop=(k == t_half - 1),
                                )
                            blk = slice(b * P, (b + 1) * P)
                            pblk = slice(bb * P, (bb + 1) * P)
                            if h == 0:
                                nc.vector.tensor_copy(out=prt[:, pblk],
                                                      in_=ps[:])
                                # trigger G1's lo-table AllGathers a few
                                # calls into G2 pass A: inputs are ready and
                                # Pool still has gather work queued behind
                                if (dh, bb) == (1, 4):
                                    flush_ags(pending)
                                continue
                            # ---- pass B: finalize block b -----------------
                            if tx0_fm is None:
                                nc.vector.tensor_tensor(
                                    out=out_fm[:, blk], in0=ps[:],
                                    in1=prt[:, pblk], op=mybir.AluOpType.add)
                            else:
                                t1 = tmpp.tile([P, P], F32, tag="t1")
                                nc.vector.scalar_tensor_tensor(
                                    out=t1[:], in0=prt[:, pblk], scalar=2.0,
                                    in1=tx0_fm[:, blk],
                                    op0=mybir.AluOpType.mult,
                                    op1=mybir.AluOpType.subtract)
                                nc.vector.scalar_tensor_tensor(
                                    out=out_fm[:, blk], in0=ps[:], scalar=2.0,
                                    in1=t1[:],
                                    op0=mybir.AluOpType.mult,
                                    op1=mybir.AluOpType.add)
                            if table is not None:
                                build_block_table(out_fm, blk, b, table,
                                                  pending)
                            if cheb is not None:
                                po = psop.tile([P, P], F32, tag="po",
                                               space="PSUM")
                                txs = (cheb["tx0"], cheb["tx1"], out_fm)
                                for k2, txk in enumerate(txs):
                                    nc.tensor.matmul(
                                        out=po[:],
                                        lhsT=wslice(cheb["wbase"] + k2),
                                        rhs=txk[:, blk],
                                        start=(k2 == 0), stop=(k2 == 2))
                                h_fm = cheb["h_fm"]
                                bc = cheb["bias_col"]
                                if cheb["relu"]:
                                    nc.scalar.activation(
                                        h_fm[:, blk], po[:],
                                        mybir.ActivationFunctionType.Relu,
                                        bias=bias_t[:, bc:bc + 1], scale=1.0)
                                else:
                                    nc.vector.tensor_scalar(
                                        out=h_fm[:, blk], in0=po[:],
                                        scalar1=bias_t[:, bc:bc + 1],
                                        scalar2=None,
                                        op0=mybir.AluOpType.add)
                                if cheb.get("table") is not None:
                                    build_block_table(h_fm, blk, b,
                                                      cheb["table"], pending)
                                if cheb.get("mlp"):
                                    pm = psop.tile([P, P], F32, tag="po",
                                                   space="PSUM")
                                    nc.tensor.matmul(out=pm[:],
                                                     lhsT=wslice(9),
                                                     rhs=h_fm[:, blk],
                                                     start=True, stop=True)
                                    h4 = nmp.tile([P, P], F16, tag="h4")
                                    nc.scalar.activation(
                                        h4[:], pm[:],
                                        mybir.ActivationFunctionType.Relu,
                                        bias=bias_t[:, 3:4], scale=1.0)
                                    p2 = psop.tile([1, P], F32, tag="po",
                                                   space="PSUM")
                                    nc.tensor.matmul(out=p2[:], lhsT=w2_ap,
                                                     rhs=h4[:],
                                                     start=True, stop=True)
                                    yo = nmp.tile([1, P], F32, tag="yo")
                                    nc.scalar.activation(
                                        yo[:], p2[:],
                                        mybir.ActivationFunctionType.Sigmoid,
                                        bias=b2_val, scale=1.0)
                                    nc.sync.dma_start(y_d[:, blk], yo[:1, :])
                        # flush G2's hi-table AllGathers at the propagate end
                        if (dh, h) == (1, 1):
                            flush_ags(pending)

            # ================= layer 1 =================
            propagate(xA, xB, tB,
                      table=(bncA[0], bncB[0], tabsA[0], tabsB[0]))  # Tx1
            propagate(tabsA[0], tabsB[0], tC, tx0_fm=tA,
                      cheb=dict(tx0=tA, tx1=tB, wbase=0, bias_col=0,
                                relu=True, h_fm=tD,
                                table=(bncA[1], bncB[1], tabsA[1], tabsB[1])))

            # ================= layer 2 =================
            propagate(tabsA[1], tabsB[1], tB,
                      table=(bncA[2], bncB[2], tabsA[2], tabsB[2]))
            propagate(tabsA[2], tabsB[2], tC, tx0_fm=tD,
                      cheb=dict(tx0=tD, tx1=tB, wbase=3, bias_col=1,
                                relu=True, h_fm=tA,
                                table=(bncA[3], bncB[3], tabsA[3], tabsB[3])))

            # ================= layer 3 + MLP head =================
            propagate(tabsA[3], tabsB[3], tB,
                      table=(bncA[4], bncB[4], tabsA[4], tabsB[4]))
            propagate(tabsA[4], tabsB[4], tC, tx0_fm=tA,
                      cheb=dict(tx0=tA, tx1=tB, wbase=6, bias_col=2,
                                relu=False, h_fm=tD, mlp=True))

    nc.finalize()
    return nc


_CACHE = {}


def kernel(x, edge_index, edge_weight, W_in, b_in, W_hid, b_hid, W_out, b_out,
           mlp_w1, mlp_b1, bn_gamma, bn_beta, bn_mean, bn_var, mlp_w2, mlp_b2,
           _trace=False):
    x = np.asarray(x, dtype=np.float32)
    t_half, idx_all, smat_all, pos = _preprocess(
        np.asarray(edge_index), np.asarray(edge_weight))

    b2_val = float(np.asarray(mlp_b2, np.float32).reshape(-1)[0])
    cache_key = (t_half, b2_val)
    if cache_key in _CACHE:
        nc = _CACHE[cache_key]
    else:
        nc = _build_program(t_half, b2_val)
        _CACHE[cache_key] = nc

    # ---- host-side tensor prep (x rows permuted to balanced positions) --
    xpad = np.zeros((N_PAD, D), dtype=np.float32)
    xpad[pos[:N_NODES]] = x
    x16_np = xpad.astype(npf16)
    xA_t = _chunk_perm_half(x16_np[:HALF])
    xB_t = _chunk_perm_half(x16_np[HALF:])

    # BN folding: y = s*(h@W1 + b1) + t -> W1' = W1*s, b1' = b1*s + t
    s = (np.asarray(bn_gamma, np.float32)
         / np.sqrt(np.asarray(bn_var, np.float32) + BN_EPS))
    t_ = np.asarray(bn_beta, np.float32) - np.asarray(bn_mean, np.float32) * s
    w1p = np.asarray(mlp_w1, np.float32) * s[None, :]
    b1p = np.asarray(mlp_b1, np.float32) * s + t_

    wts = np.zeros((P, 10 * D + 1), dtype=npf16)
    for i, W in enumerate((W_in, W_hid, W_out)):
        W = np.asarray(W, np.float32)
        for k in range(K):
            wts[:, (i * K + k) * D:(i * K + k + 1) * D] = W[k].astype(npf16)
    wts[:, 9 * D:10 * D] = w1p.astype(npf16)
    wts[:, 10 * D:10 * D + 1] = np.asarray(mlp_w2, np.float32).astype(npf16)

    biases = np.zeros((P, 4), dtype=np.float32)
    biases[:, 0] = np.asarray(b_in, np.float32)
    biases[:, 1] = np.asarray(b_hid, np.float32)
    biases[:, 2] = np.asarray(b_out, np.float32)
    biases[:, 3] = b1p

    in_maps = []
    for c in range(N_CORES):
        own = np.concatenate([
            x16_np[c * HSLAB:(c + 1) * HSLAB],
            x16_np[HALF + c * HSLAB:HALF + (c + 1) * HSLAB],
        ], axis=0)  # [6400, 128]
        in_maps.append({
            "xA": xA_t,
            "xB": xB_t,
            "x0fm": np.ascontiguousarray(own.T),
            "idx": idx_all[c],
            "smat": smat_all[c],
            "wts": wts,
            "bias": biases,
        })

    res = run_bass_kernel_spmd(nc, in_maps, list(range(N_CORES)), trace=_trace)
    y_full = np.zeros(N_PAD, dtype=np.float32)
    for c in range(N_CORES):
        yc = res.results[c]["y"][0]
        y_full[c * HSLAB:(c + 1) * HSLAB] = yc[:HSLAB]
        y_full[HALF + c * HSLAB:HALF + (c + 1) * HSLAB] = yc[HSLAB:]
    out = y_full[pos[:N_NODES], None].astype(np.float32)
    if _trace:
        kernel._last_results = res
    return out
